# revision 1
# baseline (speedup 1.0000x reference)
"""Trainium2 Bass kernel for IntegralTransform GNN message passing.

Strategy (dst-sharded, 8 cores):
  - Node space padded to 50176 = 8 * 49 * 128. Core c owns nodes
    [c*6272, (c+1)*6272) = 49 buckets of 128 nodes.
  - Host bins edges by dst bucket (stable sort), gathers pos[src]|pos[dst]
    (-> pos_enc^T stream) and x[src], and emits per-bucket chunked arrays
    padded to K chunks of 128 edges per bucket.
  - Device per bucket: L1/L2 MLP with features on partitions (weights
    stationary, bf16), per-chunk L3 uses the h2 chunk as the *stationary*
    operand so h = h2 @ Wo_perm comes out with edges on partitions
    ([128e, 256(o,i)]). DVE multiplies h by xs broadcast along o and
    group-reduces over i -> msg [128e, 16]. One-hot (iota vs dst_local)
    matmul scatters [msg | xs] into a per-bucket PSUM accumulator
    [32, 128 nodes].
  - Wo bias handled exactly at the end: out += boM^T @ xs_agg.
  - No collectives needed; host concatenates per-core [16, 6272] outputs.
"""

import numpy as np
import ml_dtypes

N_POINTS = 50000
N_PAD = 50176          # 8 * 49 * 128
N_CORES = 8
BUCKET = 128           # nodes per bucket
B_PER_CORE = 49
N_BUCKETS = N_PAD // BUCKET   # 392
CORE_NODES = B_PER_CORE * BUCKET  # 6272
IN_CH = 16
OUT_CH = 16
HID = 64
POS = 3

BF16 = ml_dtypes.bfloat16

_PROGRAM_CACHE = {}


def _build_program(K):
    """Build + compile the per-core Bass program. K = chunks per bucket."""
    import concourse.bacc as bacc
    import concourse.tile as tile
    import concourse.mybir as mybir

    f32 = mybir.dt.float32
    bf16 = mybir.dt.bfloat16
    S = K * 128  # edge slots per bucket

    nc = bacc.Bacc("TRN2", target_bir_lowering=False, debug=False)

    PT = nc.dram_tensor("PT", [6, B_PER_CORE * S], bf16, kind="ExternalInput")
    XSB = nc.dram_tensor("XSB", [B_PER_CORE, 128, K * 16], bf16, kind="ExternalInput")
    DL = nc.dram_tensor("DL", [B_PER_CORE, 128, K], f32, kind="ExternalInput")
    W1 = nc.dram_tensor("W1", [6, HID], bf16, kind="ExternalInput")
    B1 = nc.dram_tensor("B1", [HID, 1], f32, kind="ExternalInput")
    WH = nc.dram_tensor("WH", [HID, HID], bf16, kind="ExternalInput")
    BH = nc.dram_tensor("BH", [HID, 1], f32, kind="ExternalInput")
    WO = nc.dram_tensor("WO", [HID, 256], bf16, kind="ExternalInput")
    BOM = nc.dram_tensor("BOM", [32, 16], f32, kind="ExternalInput")
    OUT = nc.dram_tensor("OUT", [16, CORE_NODES], f32, kind="ExternalOutput")

    Gelu = mybir.ActivationFunctionType.Gelu
    Copy = mybir.ActivationFunctionType.Copy
    EQ = mybir.AluOpType.is_equal
    MUL = mybir.AluOpType.mult
    ADD = mybir.AluOpType.add
    X = mybir.AxisListType.X

    with tile.TileContext(nc) as tc:
        with tc.tile_pool(name="const", bufs=1) as cp, \
             tc.tile_pool(name="io", bufs=2) as io, \
             tc.tile_pool(name="work", bufs=4) as wk, \
             tc.tile_pool(name="psA", bufs=2, space="PSUM") as psA, \
             tc.tile_pool(name="psH", bufs=3, space="PSUM") as psH, \
             tc.tile_pool(name="psAcc", bufs=2, space="PSUM") as psAcc:

            # --- constants ---
            w1_t = cp.tile([6, HID], bf16)
            nc.sync.dma_start(out=w1_t[:], in_=W1[:])
            b1_t = cp.tile([HID, 1], f32)
            nc.sync.dma_start(out=b1_t[:], in_=B1[:])
            wh_t = cp.tile([HID, HID], bf16)
            nc.sync.dma_start(out=wh_t[:], in_=WH[:])
            bh_t = cp.tile([HID, 1], f32)
            nc.sync.dma_start(out=bh_t[:], in_=BH[:])
            wo_t = cp.tile([HID, 256], bf16)
            nc.sync.dma_start(out=wo_t[:], in_=WO[:])
            bom_t = cp.tile([32, 16], f32)
            nc.sync.dma_start(out=bom_t[:], in_=BOM[:])
            iota_t = cp.tile([128, 128], f32)
            nc.gpsimd.iota(iota_t[:], pattern=[[1, 128]], base=0,
                           channel_multiplier=0,
                           allow_small_or_imprecise_dtypes=True)
            comb_sb = cp.tile([32, B_PER_CORE * 128], f32)
            fin_sb = cp.tile([16, B_PER_CORE * 128], f32)

            for b in range(B_PER_CORE):
                pt_t = io.tile([6, S], bf16, tag="pt")
                nc.sync.dma_start(out=pt_t[:], in_=PT[:, b * S:(b + 1) * S])
                xsb_t = io.tile([128, K, 16], bf16, tag="xsb")
                nc.sync.dma_start(out=xsb_t[:], in_=XSB[b])
                dl_t = io.tile([128, K], f32, tag="dl")
                nc.sync.dma_start(out=dl_t[:], in_=DL[b])

                # scat holds [msg(16) | xs(16)] per chunk: [128, K, 32] f32
                scat_t = wk.tile([128, K, 32], f32, tag="scat", bufs=2)
                nc.gpsimd.tensor_copy(out=scat_t[:, :, 16:32], in_=xsb_t[:])

                # ---- L1 / L2 over the whole bucket in 512-col slices ----
                h1_t = wk.tile([HID, S], bf16, tag="h1", bufs=2)
                h2_t = wk.tile([HID, S], bf16, tag="h2", bufs=2)
                for s0 in range(0, S, 512):
                    w = min(512, S - s0)
                    p1 = psA.tile([HID, w], f32, tag="ps_mlp")
                    nc.tensor.matmul(p1[:], lhsT=w1_t[:], rhs=pt_t[:, s0:s0 + w],
                                     start=True, stop=True)
                    nc.scalar.activation(h1_t[:, s0:s0 + w], p1[:], Gelu,
                                         bias=b1_t[:], scale=1.0)
                    p2 = psA.tile([HID, w], f32, tag="ps_mlp")
                    nc.tensor.matmul(p2[:], lhsT=wh_t[:], rhs=h1_t[:, s0:s0 + w],
                                     start=True, stop=True)
                    nc.scalar.activation(h2_t[:, s0:s0 + w], p2[:], Gelu,
                                         bias=bh_t[:], scale=1.0)

                acc_t = psAcc.tile([32, 128], f32, tag="acc")
                for k in range(K):
                    hp_t = psH.tile([128, 256], f32, tag="hp")
                    nc.tensor.matmul(hp_t[:], lhsT=h2_t[:, k * 128:(k + 1) * 128],
                                     rhs=wo_t[:], start=True, stop=True)
                    xs_b = xsb_t[:, k, :].unsqueeze(1).to_broadcast([128, 16, 16])
                    prod_t = wk.tile([128, 16, 16], bf16, tag="prod", bufs=4)
                    if k % 2 == 0:
                        # ACT copies PSUM->SBUF bf16, DVE multiplies at 2x
                        hsb_t = wk.tile([128, 256], bf16, tag="hsb", bufs=3)
                        nc.scalar.activation(hsb_t[:], hp_t[:], Copy)
                        nc.vector.tensor_tensor(
                            out=prod_t[:],
                            in0=hsb_t[:].rearrange("p (o i) -> p o i", o=16, i=16),
                            in1=xs_b, op=MUL)
                    else:
                        # DVE reads PSUM directly (1x) to offload ACT
                        nc.vector.tensor_tensor(
                            out=prod_t[:],
                            in0=hp_t[:].rearrange("p (o i) -> p o i", o=16, i=16),
                            in1=xs_b, op=MUL)
                    nc.vector.tensor_reduce(out=scat_t[:, k, 0:16],
                                            in_=prod_t[:], axis=X, op=ADD)
                    oh_t = wk.tile([128, 128], f32, tag="oh", bufs=4)
                    nc.vector.tensor_scalar(out=oh_t[:], in0=iota_t[:],
                                            scalar1=dl_t[:, k:k + 1],
                                            scalar2=None, op0=EQ)
                    nc.tensor.matmul(acc_t[:], lhsT=scat_t[:, k, :], rhs=oh_t[:],
                                     start=(k == 0), stop=(k == K - 1))

                nc.scalar.activation(comb_sb[:, b * 128:(b + 1) * 128],
                                     acc_t[:], Copy)

            # ---- bias fix: out = msg + boM^T @ xs_agg ----
            # bom_t rows 0:16 are zero, rows 16:32 hold boM, so the msg half
            # of comb_sb contributes nothing to the matmul.
            for s0 in range(0, B_PER_CORE * 128, 512):
                w = min(512, B_PER_CORE * 128 - s0)
                pb = psA.tile([16, w], f32, tag="ps_mlp")
                nc.tensor.matmul(pb[:], lhsT=bom_t[:], rhs=comb_sb[:, s0:s0 + w],
                                 start=True, stop=True)
                nc.vector.tensor_tensor(out=fin_sb[:, s0:s0 + w],
                                        in0=comb_sb[0:16, s0:s0 + w], in1=pb[:],
                                        op=ADD)
            nc.sync.dma_start(out=OUT[:], in_=fin_sb[:])

    nc.compile()
    return nc


def _host_prep(x, pos, edge_index, W1, b1, Wh, bh, Wo, bo):
    """Bin edges by dst bucket, gather, pad; build per-core input maps."""
    x_flat = np.ascontiguousarray(x.reshape(-1, IN_CH).astype(np.float32))
    pos = np.ascontiguousarray(pos.astype(np.float32))
    src = np.asarray(edge_index[0], dtype=np.int64)
    dst = np.asarray(edge_index[1], dtype=np.int64)
    E = src.shape[0]

    bucket = (dst >> 7).astype(np.int32)          # 0..390
    order = np.argsort(bucket, kind="stable")
    sb = bucket[order]
    cnt = np.bincount(bucket, minlength=N_BUCKETS)
    K = int(np.max((cnt + 127) // 128))
    S = K * 128

    starts = np.zeros(N_BUCKETS, dtype=np.int64)
    starts[1:] = np.cumsum(cnt)[:-1]
    rank = np.arange(E, dtype=np.int64) - starts[sb]
    slot = sb.astype(np.int64) * S + rank          # global slot id

    e_src = src[order]
    e_dst = dst[order]

    total = N_BUCKETS * S
    PT_full = np.zeros((total, 6), dtype=np.float32)
    PT_full[slot, 0:3] = pos[e_src]
    PT_full[slot, 3:6] = pos[e_dst]
    XS_full = np.zeros((total, IN_CH), dtype=np.float32)
    XS_full[slot] = x_flat[e_src]
    DL_full = np.full(total, 999.0, dtype=np.float32)
    DL_full[slot] = (e_dst - (sb.astype(np.int64) << 7)).astype(np.float32)

    per_core = []
    core_slots = B_PER_CORE * S
    for c in range(N_CORES):
        sl = slice(c * core_slots, (c + 1) * core_slots)
        ptc = np.ascontiguousarray(PT_full[sl].T).astype(BF16)      # [6, 49*S]
        xsc = XS_full[sl].reshape(B_PER_CORE, K, 128, IN_CH)
        xsc = np.ascontiguousarray(xsc.transpose(0, 2, 1, 3)).reshape(
            B_PER_CORE, 128, K * IN_CH).astype(BF16)
        dlc = DL_full[sl].reshape(B_PER_CORE, K, 128)
        dlc = np.ascontiguousarray(dlc.transpose(0, 2, 1))          # [49,128,K]
        per_core.append({"PT": ptc, "XSB": xsc, "DL": dlc})

    # weights (shared across cores)
    W1a = np.asarray(W1, dtype=BF16)                                # [6, 64]
    b1a = np.asarray(b1, dtype=np.float32).reshape(HID, 1)
    Wha = np.asarray(Wh, dtype=BF16)                                # [64, 64]
    bha = np.asarray(bh, dtype=np.float32).reshape(HID, 1)
    WoP = np.asarray(Wo, dtype=np.float32).reshape(HID, IN_CH, OUT_CH)
    WoP = np.ascontiguousarray(WoP.transpose(0, 2, 1)).reshape(HID, 256)
    WoP = WoP.astype(BF16)                                          # [64,(o,i)]
    boM = np.zeros((32, OUT_CH), dtype=np.float32)                  # [pad+i, o]
    boM[16:32] = np.asarray(bo, dtype=np.float32).reshape(IN_CH, OUT_CH)
    shared = {"W1": W1a, "B1": b1a, "WH": Wha, "BH": bha, "WO": WoP, "BOM": boM}
    for m in per_core:
        m.update(shared)
    return K, per_core


def kernel(**inputs):
    from concourse import bass_utils

    K, in_maps = _host_prep(
        inputs["x"], inputs["pos"], inputs["edge_index"],
        inputs["W1"], inputs["b1"], inputs["Wh"], inputs["bh"],
        inputs["Wo"], inputs["bo"])

    if K not in _PROGRAM_CACHE:
        _PROGRAM_CACHE[K] = _build_program(K)
    nc = _PROGRAM_CACHE[K]

    res = bass_utils.run_bass_kernel_spmd(nc, in_maps,
                                          core_ids=list(range(N_CORES)))
    out = np.concatenate([r["OUT"] for r in res.results], axis=1)  # [16, 50176]
    out = out[:, :N_POINTS].T.astype(np.float32)                   # [50000, 16]
    return np.ascontiguousarray(out.reshape(1, N_POINTS, OUT_CH))



# revision 6
# speedup vs baseline: 1.0716x; 1.0716x over previous
"""Trainium2 Bass kernel for IntegralTransform GNN message passing.

Strategy (dst-sharded, 8 cores, V2):
  - Node space padded to 50176 = 8 * 49 * 128. Core c owns nodes
    [c*6272, (c+1)*6272) = 49 buckets of 128 nodes.
  - Host bins edges by dst bucket (stable sort), gathers pos[src]|pos[dst]
    (-> pos_enc^T stream) and x[src], and emits per-bucket chunked arrays
    padded to K chunks of 128 edges per bucket. Host also precomputes
    xs_agg[n, i] = sum_{e->n} x[src_e] so the Wo-bias term needs no
    device-side scatter.
  - Device per bucket: L1/L2 MLP with features on partitions (weights
    stationary bf16, back-to-back matmuls over 512-col slices). Per chunk,
    L3 uses the h2 chunk as the stationary operand so hp = h2c^T @ Wo_perm
    lands with edges on partitions ([128e, 256(o,i)]). DVE multiplies hp
    by xs broadcast along o -> prod bf16 (alternating: ACT copies
    PSUM->SBUF then DVE 2x-ish, or DVE reads PSUM fused). One-hot
    (iota==dst_local, bf16) matmuls scatter BOTH 128-col halves of prod
    into per-bucket PSUM accumulators acc2a/acc2b [128(o,i), 128n] --
    the i-contraction is deferred to a final per-512-slice matmul pass
    with 0/1 selection matrices, which also adds the bias term
    bo^T @ xs_agg. No collectives; host concatenates per-core outputs.
"""

import numpy as np
import ml_dtypes

N_POINTS = 50000
N_PAD = 50176          # 8 * 49 * 128
N_CORES = 8
BUCKET = 128           # nodes per bucket
B_PER_CORE = 49
N_BUCKETS = N_PAD // BUCKET   # 392
CORE_NODES = B_PER_CORE * BUCKET  # 6272
IN_CH = 16
OUT_CH = 16
HID = 64
POS = 3
PAD_DL = 300.0         # bf16-exact, != any node-local index 0..127

BF16 = ml_dtypes.bfloat16

_PROGRAM_CACHE = {}


def _build_program(K):
    """Build + compile the per-core Bass program. K = chunks per bucket."""
    import concourse.bacc as bacc
    import concourse.tile as tile
    import concourse.mybir as mybir

    f32 = mybir.dt.float32
    bf16 = mybir.dt.bfloat16
    S = K * 128  # edge slots per bucket

    nc = bacc.Bacc("TRN2", target_bir_lowering=False, debug=False)

    PT = nc.dram_tensor("PT", [6, B_PER_CORE * S], bf16, kind="ExternalInput")
    XSB = nc.dram_tensor("XSB", [B_PER_CORE, 128, K * 16], bf16, kind="ExternalInput")
    DL = nc.dram_tensor("DL", [B_PER_CORE, 128, K], f32, kind="ExternalInput")
    W1 = nc.dram_tensor("W1", [6, HID], bf16, kind="ExternalInput")
    B1 = nc.dram_tensor("B1", [HID, 1], f32, kind="ExternalInput")
    WH = nc.dram_tensor("WH", [HID, HID], bf16, kind="ExternalInput")
    BH = nc.dram_tensor("BH", [HID, 1], f32, kind="ExternalInput")
    WO = nc.dram_tensor("WO", [HID, 256], bf16, kind="ExternalInput")
    SELA = nc.dram_tensor("SELA", [128, 16], bf16, kind="ExternalInput")
    SELB = nc.dram_tensor("SELB", [128, 16], bf16, kind="ExternalInput")
    BO16 = nc.dram_tensor("BO16", [16, 16], bf16, kind="ExternalInput")
    XAGG = nc.dram_tensor("XAGG", [16, CORE_NODES], bf16, kind="ExternalInput")
    OUT = nc.dram_tensor("OUT", [16, CORE_NODES], f32, kind="ExternalOutput")

    Gelu = mybir.ActivationFunctionType.Gelu
    Copy = mybir.ActivationFunctionType.Copy
    EQ = mybir.AluOpType.is_equal
    MUL = mybir.AluOpType.mult

    # final pass: 12 slices of 512 + one of 128; slice s is ready once
    # bucket 4s+3 (or 48) has been accumulated and copied out.
    fin_slices = [(s0, min(512, CORE_NODES - s0))
                  for s0 in range(0, CORE_NODES, 512)]

    with tile.TileContext(nc) as tc:
        with tc.tile_pool(name="const", bufs=1) as cp, \
             tc.tile_pool(name="io", bufs=3) as io, \
             tc.tile_pool(name="hh", bufs=2) as hh, \
             tc.tile_pool(name="wk", bufs=4) as wk, \
             tc.tile_pool(name="ps1", bufs=2, space="PSUM") as ps1, \
             tc.tile_pool(name="psH", bufs=3, space="PSUM") as psH, \
             tc.tile_pool(name="psAcc", bufs=2, space="PSUM") as psAcc, \
             tc.tile_pool(name="psO", bufs=1, space="PSUM") as psO:

            # --- constants ---
            w1_t = cp.tile([6, HID], bf16)
            nc.sync.dma_start(out=w1_t[:], in_=W1[:])
            b1_t = cp.tile([HID, 1], f32)
            nc.sync.dma_start(out=b1_t[:], in_=B1[:])
            wh_t = cp.tile([HID, HID], bf16)
            nc.sync.dma_start(out=wh_t[:], in_=WH[:])
            bh_t = cp.tile([HID, 1], f32)
            nc.sync.dma_start(out=bh_t[:], in_=BH[:])
            wo_t = cp.tile([HID, 256], bf16)
            nc.sync.dma_start(out=wo_t[:], in_=WO[:])
            sela_t = cp.tile([128, 16], bf16)
            nc.sync.dma_start(out=sela_t[:], in_=SELA[:])
            selb_t = cp.tile([128, 16], bf16)
            nc.sync.dma_start(out=selb_t[:], in_=SELB[:])
            bo16_t = cp.tile([16, 16], bf16)
            nc.sync.dma_start(out=bo16_t[:], in_=BO16[:])
            xagg_t = cp.tile([16, CORE_NODES], bf16)
            nc.sync.dma_start(out=xagg_t[:], in_=XAGG[:])

            iota_f = cp.tile([128, 128], f32)
            nc.gpsimd.iota(iota_f[:], pattern=[[1, 128]], base=0,
                           channel_multiplier=0,
                           allow_small_or_imprecise_dtypes=True)
            iota_t = cp.tile([128, 128], bf16)
            nc.scalar.activation(iota_t[:], iota_f[:], Copy)

            comb2a = cp.tile([128, CORE_NODES], bf16)
            comb2b = cp.tile([128, CORE_NODES], bf16)
            fin_sb = cp.tile([16, CORE_NODES], f32)

            def final_slice(si):
                s0, w = fin_slices[si]
                pso = psO.tile([16, 512], f32, tag="pso")
                nc.tensor.matmul(pso[:, 0:w], lhsT=sela_t[:],
                                 rhs=comb2a[:, s0:s0 + w],
                                 start=True, stop=False)
                nc.tensor.matmul(pso[:, 0:w], lhsT=selb_t[:],
                                 rhs=comb2b[:, s0:s0 + w],
                                 start=False, stop=False)
                nc.tensor.matmul(pso[:, 0:w], lhsT=bo16_t[:],
                                 rhs=xagg_t[:, s0:s0 + w],
                                 start=False, stop=True)
                if si % 2 == 0:
                    nc.scalar.activation(fin_sb[:, s0:s0 + w], pso[:, 0:w], Copy)
                else:
                    nc.vector.tensor_copy(out=fin_sb[:, s0:s0 + w], in_=pso[:, 0:w])
                nc.sync.dma_start(out=OUT[:, s0:s0 + w], in_=fin_sb[:, s0:s0 + w])

            for b in range(B_PER_CORE):
                pt_t = io.tile([6, S], bf16, tag="pt")
                nc.sync.dma_start(out=pt_t[:], in_=PT[:, b * S:(b + 1) * S])
                xsb_t = io.tile([128, K, 16], bf16, tag="xsb")
                nc.sync.dma_start(out=xsb_t[:], in_=XSB[b])
                dl_t = io.tile([128, K], f32, tag="dl")
                nc.sync.dma_start(out=dl_t[:], in_=DL[b])

                # ---- L1 then L2, weight-stationary back-to-back MMs ----
                h1_t = hh.tile([HID, S], bf16, tag="h1")
                h2_t = hh.tile([HID, S], bf16, tag="h2")
                for s0 in range(0, S, 512):
                    w = min(512, S - s0)
                    p1 = ps1.tile([HID, 512], f32, tag="ps_mlp")
                    nc.tensor.matmul(p1[:, 0:w], lhsT=w1_t[:],
                                     rhs=pt_t[:, s0:s0 + w],
                                     start=True, stop=True)
                    nc.scalar.activation(h1_t[:, s0:s0 + w], p1[:, 0:w], Gelu,
                                         bias=b1_t[:], scale=1.0)
                for s0 in range(0, S, 512):
                    w = min(512, S - s0)
                    p2 = ps1.tile([HID, 512], f32, tag="ps_mlp")
                    nc.tensor.matmul(p2[:, 0:w], lhsT=wh_t[:],
                                     rhs=h1_t[:, s0:s0 + w],
                                     start=True, stop=True)
                    nc.scalar.activation(h2_t[:, s0:s0 + w], p2[:, 0:w], Gelu,
                                         bias=bh_t[:], scale=1.0)

                acc2 = psAcc.tile([128, 256], f32, tag="acc2")
                for k in range(K):
                    hp_t = psH.tile([128, 256], f32, tag="hp")
                    nc.tensor.matmul(hp_t[:], lhsT=h2_t[:, k * 128:(k + 1) * 128],
                                     rhs=wo_t[:], start=True, stop=True)
                    xs_b = xsb_t[:, k, :].unsqueeze(1).to_broadcast([128, 16, 16])
                    prod_t = wk.tile([128, 16, 16], bf16, tag="prod", bufs=4)
                    if k % 2 == 0:
                        hsb_t = wk.tile([128, 256], bf16, tag="hsb", bufs=2)
                        nc.scalar.activation(hsb_t[:], hp_t[:], Copy)
                        nc.vector.tensor_tensor(
                            out=prod_t[:],
                            in0=hsb_t[:].rearrange("p (o i) -> p o i", o=16, i=16),
                            in1=xs_b, op=MUL)
                    else:
                        nc.vector.tensor_tensor(
                            out=prod_t[:],
                            in0=hp_t[:].rearrange("p (o i) -> p o i", o=16, i=16),
                            in1=xs_b, op=MUL)
                    oh_t = wk.tile([128, 128], bf16, tag="oh", bufs=4)
                    nc.vector.tensor_scalar(out=oh_t[:], in0=iota_t[:],
                                            scalar1=dl_t[:, k:k + 1],
                                            scalar2=None, op0=EQ)
                    pr = prod_t[:].rearrange("p o i -> p (o i)")
                    nc.tensor.matmul(acc2[:, 0:128], lhsT=pr[:, 0:128],
                                     rhs=oh_t[:],
                                     start=(k == 0), stop=(k == K - 1))
                    nc.tensor.matmul(acc2[:, 128:256], lhsT=pr[:, 128:256],
                                     rhs=oh_t[:],
                                     start=(k == 0), stop=(k == K - 1))

                nc.scalar.activation(comb2a[:, b * 128:(b + 1) * 128],
                                     acc2[:, 0:128], Copy)
                nc.vector.tensor_copy(out=comb2b[:, b * 128:(b + 1) * 128],
                                      in_=acc2[:, 128:256])

                # final-pass slices become ready every 4 buckets
                if b % 4 == 3 and (b // 4) < 12:
                    final_slice(b // 4)
            final_slice(12)

    nc.compile()
    return nc


def _host_prep(x, pos, edge_index, W1, b1, Wh, bh, Wo, bo):
    """Bin edges by dst bucket, gather, pad; build per-core input maps."""
    x_flat = np.ascontiguousarray(x.reshape(-1, IN_CH).astype(np.float32))
    pos = np.ascontiguousarray(pos.astype(np.float32))
    src = np.asarray(edge_index[0], dtype=np.int64)
    dst = np.asarray(edge_index[1], dtype=np.int64)
    E = src.shape[0]

    bucket = (dst >> 7).astype(np.int32)          # 0..390
    order = np.argsort(bucket, kind="stable")
    sb = bucket[order]
    cnt = np.bincount(bucket, minlength=N_BUCKETS)
    K = int(np.max((cnt + 127) // 128))
    S = K * 128

    starts = np.zeros(N_BUCKETS, dtype=np.int64)
    starts[1:] = np.cumsum(cnt)[:-1]
    rank = np.arange(E, dtype=np.int64) - starts[sb]
    slot = sb.astype(np.int64) * S + rank          # global slot id

    e_src = src[order]
    e_dst = dst[order]

    total = N_BUCKETS * S
    PT_full = np.zeros((total, 6), dtype=np.float32)
    PT_full[slot, 0:3] = pos[e_src]
    PT_full[slot, 3:6] = pos[e_dst]
    XS_full = np.zeros((total, IN_CH), dtype=np.float32)
    XS_full[slot] = x_flat[e_src]
    DL_full = np.full(total, PAD_DL, dtype=np.float32)
    DL_full[slot] = (e_dst - (sb.astype(np.int64) << 7)).astype(np.float32)

    # xs_agg[n, i] = sum over edges with dst == n of x[src]  (bias path)
    xs_agg = np.zeros((N_PAD, IN_CH), dtype=np.float32)
    np.add.at(xs_agg, e_dst, x_flat[e_src])

    per_core = []
    core_slots = B_PER_CORE * S
    for c in range(N_CORES):
        sl = slice(c * core_slots, (c + 1) * core_slots)
        ptc = np.ascontiguousarray(PT_full[sl].T).astype(BF16)      # [6, 49*S]
        xsc = XS_full[sl].reshape(B_PER_CORE, K, 128, IN_CH)
        xsc = np.ascontiguousarray(xsc.transpose(0, 2, 1, 3)).reshape(
            B_PER_CORE, 128, K * IN_CH).astype(BF16)
        dlc = DL_full[sl].reshape(B_PER_CORE, K, 128)
        dlc = np.ascontiguousarray(dlc.transpose(0, 2, 1))
        xac = np.ascontiguousarray(
            xs_agg[c * CORE_NODES:(c + 1) * CORE_NODES].T).astype(BF16)
        per_core.append({"PT": ptc, "XSB": xsc, "DL": dlc, "XAGG": xac})

    # weights (shared across cores)
    W1a = np.asarray(W1, dtype=BF16)                                # [6, 64]
    b1a = np.asarray(b1, dtype=np.float32).reshape(HID, 1)
    Wha = np.asarray(Wh, dtype=BF16)                                # [64, 64]
    bha = np.asarray(bh, dtype=np.float32).reshape(HID, 1)
    WoP = np.asarray(Wo, dtype=np.float32).reshape(HID, IN_CH, OUT_CH)
    WoP = np.ascontiguousarray(WoP.transpose(0, 2, 1)).reshape(HID, 256)
    WoP = WoP.astype(BF16)                                          # [64,(o,i)]
    # selection matrices for the deferred i-contraction:
    # comb2a rows r = o*16+i for o in 0..7 -> col o; comb2b for o in 8..15
    sela = np.zeros((128, 16), dtype=np.float32)
    selb = np.zeros((128, 16), dtype=np.float32)
    for o in range(8):
        for i in range(16):
            sela[o * 16 + i, o] = 1.0
            selb[o * 16 + i, o + 8] = 1.0
    bo16 = np.asarray(bo, dtype=np.float32).reshape(IN_CH, OUT_CH)  # [i, o]
    shared = {"W1": W1a, "B1": b1a, "WH": Wha, "BH": bha, "WO": WoP,
              "SELA": sela.astype(BF16), "SELB": selb.astype(BF16),
              "BO16": bo16.astype(BF16)}
    for m in per_core:
        m.update(shared)
    return K, per_core


def kernel(**inputs):
    from concourse import bass_utils

    K, in_maps = _host_prep(
        inputs["x"], inputs["pos"], inputs["edge_index"],
        inputs["W1"], inputs["b1"], inputs["Wh"], inputs["bh"],
        inputs["Wo"], inputs["bo"])

    if K not in _PROGRAM_CACHE:
        _PROGRAM_CACHE[K] = _build_program(K)
    nc = _PROGRAM_CACHE[K]

    res = bass_utils.run_bass_kernel_spmd(nc, in_maps,
                                          core_ids=list(range(N_CORES)))
    out = np.concatenate([r["OUT"] for r in res.results], axis=1)  # [16, 50176]
    out = out[:, :N_POINTS].T.astype(np.float32)                   # [50000, 16]
    return np.ascontiguousarray(out.reshape(1, N_POINTS, OUT_CH))


# revision 12
# speedup vs baseline: 1.5088x; 1.4079x over previous
"""Trainium2 Bass kernel for IntegralTransform GNN message passing.

Strategy (dst-sharded, 8 cores, V3):
  - Node space padded to 50176 = 8 * 49 * 128. Core c owns nodes
    [c*6272, (c+1)*6272) = 49 buckets of 128 nodes. Host bins edges by dst
    bucket (stable sort), gathers pos[src]|pos[dst] and x[src], pads each
    bucket to K chunks of 128 edges (K forced even). Host precomputes
    xs_agg[n,i] (bias path) and the per-chunk one-hot scatter matrices
    (bf16), so the device builds nothing per-edge except the MLP.
  - Edge stream is laid out two-chunks-deep: even chunks live on SBUF
    partitions 0-63, odd chunks on 64-127. L1/L2 run as two concurrent
    PE sub-tiles (col/row groups), so gelu processes 128 partitions per
    op (half the ACT time). L3 (hp = h2c^T @ Wo_perm) runs as concurrent
    row-group pairs writing quads into one [128,1024] PSUM pair-bank.
  - DVE multiplies each hp quad by xs (broadcast along o) in ONE op
    straight out of PSUM -> prod bf16. One-hot matmuls scatter both
    128-col halves of each prod chunk into a single per-bucket PSUM
    accumulation group acc2 [128(o,i), 2x128n packed in one bank].
    The i-contraction and Wo-bias (bo^T @ xs_agg) are deferred to a
    final per-512-slice matmul pass with 0/1 selection matrices,
    interleaved every 4 buckets. No collectives; host concatenates
    per-core [16, 6272] outputs.
"""

import numpy as np
import ml_dtypes

N_POINTS = 50000
N_PAD = 50176          # 8 * 49 * 128
N_CORES = 8
BUCKET = 128           # nodes per bucket
B_PER_CORE = 49
N_BUCKETS = N_PAD // BUCKET   # 392
CORE_NODES = B_PER_CORE * BUCKET  # 6272
IN_CH = 16
OUT_CH = 16
HID = 64
POS = 3

BF16 = ml_dtypes.bfloat16

_PROGRAM_CACHE = {}


def _build_program(K):
    """Build + compile the per-core Bass program. K = chunks per bucket
    (even)."""
    import concourse.bacc as bacc
    import concourse.tile as tile
    import concourse.mybir as mybir

    assert K % 2 == 0
    f32 = mybir.dt.float32
    bf16 = mybir.dt.bfloat16
    S = K * 128   # edge slots per bucket
    H = S // 2    # columns per partition-half in the two-deep layout

    nc = bacc.Bacc("TRN2", target_bir_lowering=False, debug=False)

    PT = nc.dram_tensor("PT", [12, B_PER_CORE * H], bf16, kind="ExternalInput")
    W12 = nc.dram_tensor("W12", [128, HID], bf16, kind="ExternalInput")
    XSB = nc.dram_tensor("XSB", [B_PER_CORE, 128, K * 16], bf16, kind="ExternalInput")
    OH = nc.dram_tensor("OH", [B_PER_CORE, 128, K * 128], bf16, kind="ExternalInput")
    W1 = nc.dram_tensor("W1", [6, HID], bf16, kind="ExternalInput")
    B1 = nc.dram_tensor("B1", [128, 1], f32, kind="ExternalInput")
    WH2 = nc.dram_tensor("WH2", [128, HID], bf16, kind="ExternalInput")
    BH = nc.dram_tensor("BH", [128, 1], f32, kind="ExternalInput")
    WO2 = nc.dram_tensor("WO2", [128, 256], bf16, kind="ExternalInput")
    SELA = nc.dram_tensor("SELA", [128, 16], bf16, kind="ExternalInput")
    SELB = nc.dram_tensor("SELB", [128, 16], bf16, kind="ExternalInput")
    BO16 = nc.dram_tensor("BO16", [16, 16], bf16, kind="ExternalInput")
    XAGG = nc.dram_tensor("XAGG", [16, CORE_NODES], bf16, kind="ExternalInput")
    OUT = nc.dram_tensor("OUT", [16, CORE_NODES], f32, kind="ExternalOutput")

    Gelu = mybir.ActivationFunctionType.Gelu
    Copy = mybir.ActivationFunctionType.Copy
    MUL = mybir.AluOpType.mult

    fin_slices = [(s0, min(512, CORE_NODES - s0))
                  for s0 in range(0, CORE_NODES, 512)]

    # chunk groups of 2 for pair-batched hp/mult (one PSUM bank each)
    groups = [list(range(g, min(g + 2, K))) for g in range(0, K, 2)]

    with tile.TileContext(nc) as tc:
        with tc.tile_pool(name="const", bufs=1) as cp, \
             tc.tile_pool(name="io", bufs=2) as io, \
             tc.tile_pool(name="hh", bufs=2) as hh, \
             tc.tile_pool(name="wk", bufs=3) as wk, \
             tc.tile_pool(name="psMLP", bufs=2, space="PSUM") as psMLP, \
             tc.tile_pool(name="psH", bufs=2, space="PSUM") as psH, \
             tc.tile_pool(name="psAcc", bufs=2, space="PSUM") as psAcc:

            # --- constants ---
            w1_t = cp.tile([6, HID], bf16)
            nc.sync.dma_start(out=w1_t[:], in_=W1[:])
            w12_t = cp.tile([128, HID], bf16)
            nc.sync.dma_start(out=w12_t[:], in_=W12[:])
            b1_t = cp.tile([128, 1], f32)
            nc.sync.dma_start(out=b1_t[:], in_=B1[:])
            wh2_t = cp.tile([128, HID], bf16)
            nc.sync.dma_start(out=wh2_t[:], in_=WH2[:])
            bh_t = cp.tile([128, 1], f32)
            nc.sync.dma_start(out=bh_t[:], in_=BH[:])
            wo2_t = cp.tile([128, 256], bf16)
            nc.sync.dma_start(out=wo2_t[:], in_=WO2[:])
            sela_t = cp.tile([128, 16], bf16)
            nc.sync.dma_start(out=sela_t[:], in_=SELA[:])
            selb_t = cp.tile([128, 16], bf16)
            nc.sync.dma_start(out=selb_t[:], in_=SELB[:])
            bo16_t = cp.tile([16, 16], bf16)
            nc.sync.dma_start(out=bo16_t[:], in_=BO16[:])
            xagg_t = cp.tile([16, CORE_NODES], bf16)
            nc.sync.dma_start(out=xagg_t[:], in_=XAGG[:])

            comb2a = cp.tile([128, CORE_NODES], bf16)
            comb2b = cp.tile([128, CORE_NODES], bf16)
            fin_sb = cp.tile([16, CORE_NODES], f32)

            def final_slice(si):
                s0, w = fin_slices[si]
                pso = psMLP.tile([128, 512], f32, tag="ps_mlp")
                nc.tensor.matmul(pso[0:16, 0:w], lhsT=sela_t[:],
                                 rhs=comb2a[:, s0:s0 + w],
                                 start=True, stop=False)
                nc.tensor.matmul(pso[0:16, 0:w], lhsT=selb_t[:],
                                 rhs=comb2b[:, s0:s0 + w],
                                 start=False, stop=False)
                nc.tensor.matmul(pso[0:16, 0:w], lhsT=bo16_t[:],
                                 rhs=xagg_t[:, s0:s0 + w],
                                 start=False, stop=True)
                if si % 2 == 0:
                    nc.scalar.activation(fin_sb[:, s0:s0 + w],
                                         pso[0:16, 0:w], Copy)
                else:
                    nc.vector.tensor_copy(out=fin_sb[:, s0:s0 + w],
                                          in_=pso[0:16, 0:w])
                nc.sync.dma_start(out=OUT[:, s0:s0 + w],
                                  in_=fin_sb[:, s0:s0 + w])

            for b in range(B_PER_CORE):
                pt_t = io.tile([128, H], bf16, tag="pt")
                nc.sync.dma_start(out=pt_t[0:6, :], in_=PT[0:6, b * H:(b + 1) * H])
                nc.sync.dma_start(out=pt_t[64:70, :], in_=PT[6:12, b * H:(b + 1) * H])
                xsb_t = io.tile([128, K, 16], bf16, tag="xsb")
                nc.sync.dma_start(out=xsb_t[:], in_=XSB[b])
                oh_t = io.tile([128, K * 128], bf16, tag="oh")
                nc.sync.dma_start(out=oh_t[:], in_=OH[b])

                # ---- L1: two concurrent col-group tiles, gelu on 128p ----
                h1_t = hh.tile([128, H], bf16, tag="h1")
                h2_t = hh.tile([128, H], bf16, tag="h2")
                h2o_t = hh.tile([64, H], bf16, tag="h2o")
                for s0 in range(0, H, 512):
                    w = min(512, H - s0)
                    p1 = psMLP.tile([128, 512], f32, tag="ps_mlp")
                    nc.tensor.matmul(p1[0:64, 0:w], lhsT=w12_t[0:6, :],
                                     rhs=pt_t[0:6, s0:s0 + w],
                                     start=True, stop=True)
                    nc.tensor.matmul(p1[64:128, 0:w], lhsT=w12_t[64:70, :],
                                     rhs=pt_t[64:70, s0:s0 + w],
                                     start=True, stop=True)
                    nc.scalar.activation(h1_t[:, s0:s0 + w], p1[:, 0:w], Gelu,
                                         bias=b1_t[:], scale=1.0)
                # ---- L2: two concurrent (row,col) diagonal tiles ----
                for s0 in range(0, H, 512):
                    w = min(512, H - s0)
                    p2 = psMLP.tile([128, 512], f32, tag="ps_mlp")
                    nc.tensor.matmul(p2[0:64, 0:w], lhsT=wh2_t[0:64, :],
                                     rhs=h1_t[0:64, s0:s0 + w],
                                     start=True, stop=True)
                    nc.tensor.matmul(p2[64:128, 0:w], lhsT=wh2_t[64:128, :],
                                     rhs=h1_t[64:128, s0:s0 + w],
                                     start=True, stop=True)
                    nc.scalar.activation(h2_t[:, s0:s0 + w], p2[:, 0:w], Gelu,
                                         bias=bh_t[:], scale=1.0)
                nc.sync.dma_start(out=h2o_t[:], in_=h2_t[64:128, :])

                # ---- per-group hp quad + mult + (skewed) scatter ----
                acc2a = psAcc.tile([128, 128], f32, tag="acc2a")
                acc2b = psAcc.tile([128, 128], f32, tag="acc2b")

                def emit_scatter(G, prod4, first, last):
                    pr = prod4[:].rearrange("p c o i -> p (c o i)")
                    for idx, c in enumerate(G):
                        q = idx * 256
                        nc.tensor.matmul(
                            acc2a[:], lhsT=pr[:, q:q + 128],
                            rhs=oh_t[:, c * 128:(c + 1) * 128],
                            start=(first and idx == 0),
                            stop=(last and idx == len(G) - 1))
                        nc.tensor.matmul(
                            acc2b[:], lhsT=pr[:, q + 128:q + 256],
                            rhs=oh_t[:, c * 128:(c + 1) * 128],
                            start=(first and idx == 0),
                            stop=(last and idx == len(G) - 1))

                for gi, G in enumerate(groups):
                    hp4 = psH.tile([128, 512], f32, tag="hp4")
                    for idx, c in enumerate(G):
                        m = c // 2
                        if c % 2 == 0:
                            lhsT = h2_t[0:64, m * 128:(m + 1) * 128]
                        else:
                            lhsT = h2o_t[:, m * 128:(m + 1) * 128]
                        nc.tensor.matmul(hp4[:, idx * 256:(idx + 1) * 256],
                                         lhsT=lhsT, rhs=wo2_t[0:64, :],
                                         start=True, stop=True)
                    n = len(G)
                    prod4 = wk.tile([128, 2, 16, 16], bf16, tag="prod")
                    xs_b = xsb_t[:, G[0]:G[0] + n, :].unsqueeze(2).to_broadcast(
                        [128, n, 16, 16])
                    nc.vector.tensor_tensor(
                        out=prod4[:, 0:n],
                        in0=hp4[:, 0:n * 256].rearrange(
                            "p (c o i) -> p c o i", c=n, o=16, i=16),
                        in1=xs_b, op=MUL)
                    emit_scatter(G, prod4, gi == 0,
                                 gi == len(groups) - 1)

                nc.scalar.activation(comb2a[:, b * 128:(b + 1) * 128],
                                     acc2a[:], Copy)
                nc.vector.tensor_copy(out=comb2b[:, b * 128:(b + 1) * 128],
                                      in_=acc2b[:])

                if b % 4 == 3 and (b // 4) < 12:
                    final_slice(b // 4)
            final_slice(12)

    nc.compile()
    return nc


def _host_prep(x, pos, edge_index, W1, b1, Wh, bh, Wo, bo):
    """Bin edges by dst bucket, gather, pad; build per-core input maps."""
    x_flat = np.ascontiguousarray(x.reshape(-1, IN_CH).astype(np.float32))
    pos = np.ascontiguousarray(pos.astype(np.float32))
    src = np.asarray(edge_index[0], dtype=np.int64)
    dst = np.asarray(edge_index[1], dtype=np.int64)
    E = src.shape[0]

    bucket = (dst >> 7).astype(np.int32)          # 0..390
    order = np.argsort(bucket, kind="stable")
    sb = bucket[order]
    cnt = np.bincount(bucket, minlength=N_BUCKETS)
    K = int(np.max((cnt + 127) // 128))
    K += K % 2                                     # force even
    S = K * 128

    starts = np.zeros(N_BUCKETS, dtype=np.int64)
    starts[1:] = np.cumsum(cnt)[:-1]
    rank = np.arange(E, dtype=np.int64) - starts[sb]
    slot = sb.astype(np.int64) * S + rank          # global slot id

    e_src = src[order]
    e_dst = dst[order]

    total = N_BUCKETS * S
    PT_full = np.zeros((total, 6), dtype=np.float32)
    PT_full[slot, 0:3] = pos[e_src]
    PT_full[slot, 3:6] = pos[e_dst]
    XS_full = np.zeros((total, IN_CH), dtype=np.float32)
    XS_full[slot] = x_flat[e_src]
    dl = (e_dst - (sb.astype(np.int64) << 7)).astype(np.int64)
    OH_full = np.zeros((total, BUCKET), dtype=BF16)
    OH_full[slot, dl] = 1

    # xs_agg[n, i] = sum over edges with dst == n of x[src]  (bias path)
    xs_agg = np.zeros((N_PAD, IN_CH), dtype=np.float32)
    np.add.at(xs_agg, e_dst, x_flat[e_src])

    per_core = []
    core_slots = B_PER_CORE * S
    for c in range(N_CORES):
        sl = slice(c * core_slots, (c + 1) * core_slots)
        # PT: [12, B*S/2] two-chunk-deep: rows 0-5 even chunks, 6-11 odd
        ptc = PT_full[sl].reshape(B_PER_CORE, K, 128, 6)
        pte = ptc[:, 0::2].reshape(B_PER_CORE, K // 2 * 128, 6)
        pto = ptc[:, 1::2].reshape(B_PER_CORE, K // 2 * 128, 6)
        ptd = np.concatenate([pte, pto], axis=2)       # [B, S/2, 12]
        ptd = np.ascontiguousarray(
            ptd.transpose(2, 0, 1)).reshape(12, B_PER_CORE * (S // 2))
        xsc = XS_full[sl].reshape(B_PER_CORE, K, 128, IN_CH)
        xsc = np.ascontiguousarray(xsc.transpose(0, 2, 1, 3)).reshape(
            B_PER_CORE, 128, K * IN_CH).astype(BF16)
        ohc = OH_full[sl].reshape(B_PER_CORE, K, 128, BUCKET)
        ohc = np.ascontiguousarray(ohc.transpose(0, 2, 1, 3)).reshape(
            B_PER_CORE, 128, K * BUCKET)
        xac = np.ascontiguousarray(
            xs_agg[c * CORE_NODES:(c + 1) * CORE_NODES].T).astype(BF16)
        per_core.append({"PT": ptd.astype(BF16), "XSB": xsc, "OH": ohc,
                         "XAGG": xac})

    # weights (shared across cores)
    W1a = np.asarray(W1, dtype=BF16)                                # [6, 64]
    W12 = np.zeros((128, HID), dtype=BF16)
    W12[0:6] = W1a
    W12[64:70] = W1a
    b1a = np.tile(np.asarray(b1, dtype=np.float32).reshape(HID, 1), (2, 1))
    Wha = np.asarray(Wh, dtype=BF16)                                # [64, 64]
    Wh2 = np.vstack([Wha, Wha])                                     # [128, 64]
    bha = np.tile(np.asarray(bh, dtype=np.float32).reshape(HID, 1), (2, 1))
    WoP = np.asarray(Wo, dtype=np.float32).reshape(HID, IN_CH, OUT_CH)
    WoP = np.ascontiguousarray(WoP.transpose(0, 2, 1)).reshape(HID, 256)
    WoP = WoP.astype(BF16)                                          # [64,(o,i)]
    Wo2 = np.vstack([WoP, WoP])                                     # [128, 256]
    sela = np.zeros((128, 16), dtype=np.float32)
    selb = np.zeros((128, 16), dtype=np.float32)
    for o in range(8):
        for i in range(16):
            sela[o * 16 + i, o] = 1.0
            selb[o * 16 + i, o + 8] = 1.0
    bo16 = np.asarray(bo, dtype=np.float32).reshape(IN_CH, OUT_CH)  # [i, o]
    shared = {"W1": W1a, "W12": W12, "B1": b1a, "WH2": Wh2, "BH": bha, "WO2": Wo2,
              "SELA": sela.astype(BF16), "SELB": selb.astype(BF16),
              "BO16": bo16.astype(BF16)}
    for m in per_core:
        m.update(shared)
    return K, per_core


def kernel(**inputs):
    from concourse import bass_utils

    K, in_maps = _host_prep(
        inputs["x"], inputs["pos"], inputs["edge_index"],
        inputs["W1"], inputs["b1"], inputs["Wh"], inputs["bh"],
        inputs["Wo"], inputs["bo"])

    if K not in _PROGRAM_CACHE:
        _PROGRAM_CACHE[K] = _build_program(K)
    nc = _PROGRAM_CACHE[K]

    res = bass_utils.run_bass_kernel_spmd(nc, in_maps,
                                          core_ids=list(range(N_CORES)))
    out = np.concatenate([r["OUT"] for r in res.results], axis=1)  # [16, 50176]
    out = out[:, :N_POINTS].T.astype(np.float32)                   # [50000, 16]
    return np.ascontiguousarray(out.reshape(1, N_POINTS, OUT_CH))


# revision 16
# speedup vs baseline: 2.7479x; 1.8212x over previous
"""Trainium2 Bass kernel for IntegralTransform GNN message passing.

Strategy (dst-sharded, 8 cores, V5):
  - Node space padded to 50176 = 8 * 49 * 128. Core c owns nodes
    [c*6272, (c+1)*6272) = 49 buckets of 128 nodes. Host bins edges by dst
    bucket (stable sort) and sorts each core's buckets by edge count
    (descending) into 49 SLOTS; slot b's chunk count k_b is the max across
    the 8 cores, so one SPMD program fits all cores with minimal padding
    (~814 chunks/core vs 882 fixed). Host gathers pos[src]|pos[dst] and
    x[src] per slot, precomputes the per-chunk one-hot scatter matrices
    (bf16), and adds the Wo-bias term (xs_agg @ bo) after the device
    returns the pure message aggregation.
  - Edge stream is laid out two-chunks-deep: even chunks live on SBUF
    partitions 0-63, odd chunks on 64-127. L1/L2 run as two concurrent
    PE sub-tiles (col/row groups), so gelu processes 128 partitions per
    op. L3 runs one matmul per chunk PAIR: the two-deep h2 column block
    [128,128] is exactly the stacked operand for a block-diagonal
    rhs [[Wo,0],[0,Wo]] [128,512], yielding hp for both chunks in one
    [128,512] PSUM bank (one LDWEIGHTS per two chunks). Odd slot sizes
    get a single-chunk tail matmul.
  - DVE multiplies each hp pair by xs (broadcast along o) in ONE op
    straight out of PSUM -> prod bf16 [128, 2, 16, 16]. Scatter is
    transposed: acc2T[n, (o,i)] += oh_c^T @ prod_c -- one N=256 matmul
    and one LDWEIGHTS (the one-hot) per chunk, a single per-slot
    accumulation group in one PSUM bank. A per-slot DVE tensor_reduce
    over i produces out[n, o] directly; one [128, 49*16] DMA returns it.
    No collectives; host un-permutes slots, concatenates, adds the bias.
"""

import numpy as np
import ml_dtypes

N_POINTS = 50000
N_PAD = 50176          # 8 * 49 * 128
N_CORES = 8
BUCKET = 128           # nodes per bucket
B_PER_CORE = 49
N_BUCKETS = N_PAD // BUCKET   # 392
CORE_NODES = B_PER_CORE * BUCKET  # 6272
IN_CH = 16
OUT_CH = 16
HID = 64
POS = 3

BF16 = ml_dtypes.bfloat16

_PROGRAM_CACHE = {}


def _build_program(ks):
    """Build + compile the per-core Bass program.

    ks = tuple of 49 per-slot chunk counts (shared across cores)."""
    import concourse.bacc as bacc
    import concourse.tile as tile
    import concourse.mybir as mybir

    f32 = mybir.dt.float32
    bf16 = mybir.dt.bfloat16

    nblks = [(k + 1) // 2 for k in ks]       # 128-col blocks in two-deep layout
    hoffs = np.concatenate([[0], np.cumsum([n * 128 for n in nblks])])
    coffs = np.concatenate([[0], np.cumsum(ks)])
    total_half = int(hoffs[-1])
    total_chunks = int(coffs[-1])
    Kmax = max(ks)
    Hmax = max(nblks) * 128

    nc = bacc.Bacc("TRN2", target_bir_lowering=False, debug=False)

    PT = nc.dram_tensor("PT", [12, total_half], bf16, kind="ExternalInput")
    XSB = nc.dram_tensor("XSB", [128, total_chunks * 16], bf16,
                         kind="ExternalInput")
    OH = nc.dram_tensor("OH", [128, total_chunks * 128], bf16,
                        kind="ExternalInput")
    W12 = nc.dram_tensor("W12", [128, HID], bf16, kind="ExternalInput")
    B1 = nc.dram_tensor("B1", [128, 1], f32, kind="ExternalInput")
    WH2 = nc.dram_tensor("WH2", [128, HID], bf16, kind="ExternalInput")
    BH = nc.dram_tensor("BH", [128, 1], f32, kind="ExternalInput")
    WOD = nc.dram_tensor("WOD", [128, 512], bf16, kind="ExternalInput")
    OUT = nc.dram_tensor("OUT", [128, B_PER_CORE * 16], f32,
                         kind="ExternalOutput")

    Gelu = mybir.ActivationFunctionType.Gelu
    MUL = mybir.AluOpType.mult
    ADD = mybir.AluOpType.add
    X = mybir.AxisListType.X

    with tile.TileContext(nc) as tc:
        with tc.tile_pool(name="const", bufs=1) as cp, \
             tc.tile_pool(name="io", bufs=3) as io, \
             tc.tile_pool(name="hh", bufs=2) as hh, \
             tc.tile_pool(name="wk", bufs=4) as wk, \
             tc.tile_pool(name="psMLP", bufs=3, space="PSUM") as psMLP, \
             tc.tile_pool(name="psH", bufs=3, space="PSUM") as psH, \
             tc.tile_pool(name="psAcc", bufs=2, space="PSUM") as psAcc:

            # --- constants ---
            w12_t = cp.tile([128, HID], bf16)
            nc.sync.dma_start(out=w12_t[:], in_=W12[:])
            b1_t = cp.tile([128, 1], f32)
            nc.sync.dma_start(out=b1_t[:], in_=B1[:])
            wh2_t = cp.tile([128, HID], bf16)
            nc.sync.dma_start(out=wh2_t[:], in_=WH2[:])
            bh_t = cp.tile([128, 1], f32)
            nc.sync.dma_start(out=bh_t[:], in_=BH[:])
            wod_t = cp.tile([128, 512], bf16)
            nc.sync.dma_start(out=wod_t[:], in_=WOD[:])

            fin_t = cp.tile([128, B_PER_CORE * 16], f32)

            for b in range(B_PER_CORE):
                k = ks[b]
                npairs, tail = k // 2, k % 2
                nblk = npairs + tail
                Hb = nblk * 128
                ho = int(hoffs[b])
                co = int(coffs[b])

                pt_t = io.tile([128, Hmax], bf16, tag="pt")
                nc.sync.dma_start(out=pt_t[0:6, 0:Hb],
                                  in_=PT[0:6, ho:ho + Hb])
                nc.sync.dma_start(out=pt_t[64:70, 0:Hb],
                                  in_=PT[6:12, ho:ho + Hb])
                xsb_t = io.tile([128, Kmax, 16], bf16, tag="xsb")
                nc.sync.dma_start(out=xsb_t[:, 0:k, :],
                                  in_=XSB[:, co * 16:(co + k) * 16])
                oh_t = io.tile([128, Kmax * 128], bf16, tag="oh")
                nc.sync.dma_start(out=oh_t[:, 0:k * 128],
                                  in_=OH[:, co * 128:(co + k) * 128])

                # ---- L1: two concurrent col-group tiles, gelu on 128p ----
                h1_t = hh.tile([128, Hmax], bf16, tag="h1")
                h2_t = hh.tile([128, Hmax], bf16, tag="h2")
                for s0 in range(0, Hb, 512):
                    w = min(512, Hb - s0)
                    p1 = psMLP.tile([128, 512], f32, tag="ps_mlp")
                    nc.tensor.matmul(p1[0:64, 0:w], lhsT=w12_t[0:6, :],
                                     rhs=pt_t[0:6, s0:s0 + w],
                                     start=True, stop=True)
                    nc.tensor.matmul(p1[64:128, 0:w], lhsT=w12_t[64:70, :],
                                     rhs=pt_t[64:70, s0:s0 + w],
                                     start=True, stop=True)
                    nc.scalar.activation(h1_t[:, s0:s0 + w], p1[:, 0:w], Gelu,
                                         bias=b1_t[:], scale=1.0)
                # ---- L2: two concurrent (row,col) diagonal tiles ----
                for s0 in range(0, Hb, 512):
                    w = min(512, Hb - s0)
                    p2 = psMLP.tile([128, 512], f32, tag="ps_mlp")
                    nc.tensor.matmul(p2[0:64, 0:w], lhsT=wh2_t[0:64, :],
                                     rhs=h1_t[0:64, s0:s0 + w],
                                     start=True, stop=True)
                    nc.tensor.matmul(p2[64:128, 0:w], lhsT=wh2_t[64:128, :],
                                     rhs=h1_t[64:128, s0:s0 + w],
                                     start=True, stop=True)
                    nc.scalar.activation(h2_t[:, s0:s0 + w], p2[:, 0:w], Gelu,
                                         bias=bh_t[:], scale=1.0)

                # ---- per-pair hp (block-diag) + mult + skewed scatter ----
                acc2 = psAcc.tile([128, 256], f32, tag="acc2")
                prev = None

                def emit_scatter(chunks, prod2, first, last):
                    pr = prod2[:].rearrange("p c o i -> p (c o i)")
                    for idx, c in enumerate(chunks):
                        nc.tensor.matmul(
                            acc2[:], lhsT=oh_t[:, c * 128:(c + 1) * 128],
                            rhs=pr[:, idx * 256:(idx + 1) * 256],
                            start=(first and idx == 0),
                            stop=(last and idx == len(chunks) - 1))

                for m in range(nblk):
                    is_tail = (tail == 1 and m == nblk - 1)
                    hpP = psH.tile([128, 512], f32, tag="hpP")
                    prod2 = wk.tile([128, 2, 16, 16], bf16, tag="prod")
                    if not is_tail:
                        nc.tensor.matmul(hpP[:],
                                         lhsT=h2_t[:, m * 128:(m + 1) * 128],
                                         rhs=wod_t[:], start=True, stop=True)
                        xs_b = xsb_t[:, 2 * m:2 * m + 2, :].unsqueeze(
                            2).to_broadcast([128, 2, 16, 16])
                        nc.vector.tensor_tensor(
                            out=prod2[:],
                            in0=hpP[:].rearrange("p (c o i) -> p c o i",
                                                 c=2, o=16, i=16),
                            in1=xs_b, op=MUL)
                        chunks = [2 * m, 2 * m + 1]
                    else:
                        nc.tensor.matmul(hpP[:, 0:256],
                                         lhsT=h2_t[0:64, m * 128:(m + 1) * 128],
                                         rhs=wod_t[0:64, 0:256],
                                         start=True, stop=True)
                        xs_b = xsb_t[:, k - 1:k, :].unsqueeze(
                            2).to_broadcast([128, 1, 16, 16])
                        nc.vector.tensor_tensor(
                            out=prod2[:, 0:1],
                            in0=hpP[:, 0:256].rearrange(
                                "p (c o i) -> p c o i", c=1, o=16, i=16),
                            in1=xs_b, op=MUL)
                        chunks = [k - 1]
                    if prev is not None:
                        emit_scatter(prev[0], prev[1], prev[2], False)
                    prev = (chunks, prod2, m == 0)
                emit_scatter(prev[0], prev[1], prev[2], True)

                # out[n, o] = sum_i acc2T[n, (o,i)]
                nc.vector.tensor_reduce(
                    out=fin_t[:, b * 16:(b + 1) * 16],
                    in_=acc2[:].rearrange("p (o i) -> p o i", o=16, i=16),
                    axis=X, op=ADD)
                if b in (15, 31, 43, B_PER_CORE - 1):
                    lo = {15: 0, 31: 16, 43: 32, B_PER_CORE - 1: 44}[b] * 16
                    hi = (b + 1) * 16
                    nc.sync.dma_start(out=OUT[:, lo:hi],
                                      in_=fin_t[:, lo:hi])

    nc.compile()
    return nc


def _host_prep(x, pos, edge_index, W1, b1, Wh, bh, Wo, bo):
    """Bin edges by dst bucket, sort buckets into slots, gather, pad."""
    x_flat = np.ascontiguousarray(x.reshape(-1, IN_CH).astype(np.float32))
    pos = np.ascontiguousarray(pos.astype(np.float32))
    src = np.asarray(edge_index[0], dtype=np.int64)
    dst = np.asarray(edge_index[1], dtype=np.int64)
    E = src.shape[0]

    bucket = (dst >> 7).astype(np.int64)          # 0..391
    order = np.argsort(bucket, kind="stable")     # edge ids sorted by bucket
    cnt = np.bincount(bucket, minlength=N_BUCKETS)
    starts = np.zeros(N_BUCKETS, dtype=np.int64)
    starts[1:] = np.cumsum(cnt)[:-1]

    cnt_pc = cnt.reshape(N_CORES, B_PER_CORE)
    perms = np.argsort(-cnt_pc, axis=1, kind="stable")   # slot -> local bucket
    sorted_cnt = np.take_along_axis(cnt_pc, perms, axis=1)
    slot_max = sorted_cnt.max(axis=0)
    ks = tuple(int(v) for v in np.maximum(1, np.ceil(slot_max / 128))
               .astype(np.int64))

    nblks = [(k + 1) // 2 for k in ks]
    hoffs = np.concatenate([[0], np.cumsum([n * 128 for n in nblks])])
    coffs = np.concatenate([[0], np.cumsum(ks)])
    total_half = int(hoffs[-1])
    total_chunks = int(coffs[-1])

    # host-side bias term
    e_src_all = src[order]
    e_dst_all = dst[order]
    xs_agg = np.zeros((N_PAD, IN_CH), dtype=np.float32)
    np.add.at(xs_agg, e_dst_all, x_flat[e_src_all])
    bo16 = np.asarray(bo, dtype=np.float32).reshape(IN_CH, OUT_CH)
    bias_full = xs_agg @ bo16                      # [N_PAD, 16]

    per_core = []
    for c in range(N_CORES):
        PT2 = np.zeros((12, total_half), dtype=np.float32)
        XS2 = np.zeros((128, total_chunks, 16), dtype=np.float32)
        OH2 = np.zeros((128, total_chunks, 128), dtype=BF16)
        for b in range(B_PER_CORE):
            k = ks[b]
            nblk = nblks[b]
            ho = int(hoffs[b])
            g = c * B_PER_CORE + int(perms[c][b])  # global bucket id
            n = int(cnt_pc[c][perms[c][b]])
            if n == 0:
                continue
            eids = order[starts[g]:starts[g] + n]
            es, ed = src[eids], dst[eids]
            pe6 = np.concatenate([pos[es], pos[ed]], axis=1)   # [n, 6]
            xse = x_flat[es]                                   # [n, 16]
            dl = (ed - (g << 7)).astype(np.int64)
            ch = np.arange(n) // 128                           # chunk in slot
            rw = np.arange(n) % 128                            # row (edge lane)
            # PT two-deep: chunk 2m -> rows 0-5 block m; 2m+1 -> rows 6-11
            colh = (ch // 2) * 128 + rw
            hi = (ch % 2) * 6
            PT2[hi, ho + colh] = pe6[:, 0]
            PT2[hi + 1, ho + colh] = pe6[:, 1]
            PT2[hi + 2, ho + colh] = pe6[:, 2]
            PT2[hi + 3, ho + colh] = pe6[:, 3]
            PT2[hi + 4, ho + colh] = pe6[:, 4]
            PT2[hi + 5, ho + colh] = pe6[:, 5]
            XS2[rw, int(coffs[b]) + ch] = xse
            OH2[rw, int(coffs[b]) + ch, dl] = 1
        per_core.append({
            "PT": PT2.astype(BF16),
            "XSB": np.ascontiguousarray(XS2.reshape(128, total_chunks * 16)
                                        ).astype(BF16),
            "OH": np.ascontiguousarray(OH2.reshape(128, total_chunks * 128)),
        })

    # weights (shared across cores)
    W1a = np.asarray(W1, dtype=BF16)                                # [6, 64]
    W12 = np.zeros((128, HID), dtype=BF16)
    W12[0:6] = W1a
    W12[64:70] = W1a
    b1a = np.tile(np.asarray(b1, dtype=np.float32).reshape(HID, 1), (2, 1))
    Wha = np.asarray(Wh, dtype=BF16)                                # [64, 64]
    Wh2 = np.vstack([Wha, Wha])                                     # [128, 64]
    bha = np.tile(np.asarray(bh, dtype=np.float32).reshape(HID, 1), (2, 1))
    WoP = np.asarray(Wo, dtype=np.float32).reshape(HID, IN_CH, OUT_CH)
    WoP = np.ascontiguousarray(WoP.transpose(0, 2, 1)).reshape(HID, 256)
    WoP = WoP.astype(BF16)                                          # [64,(o,i)]
    WoD = np.zeros((128, 512), dtype=BF16)
    WoD[0:64, 0:256] = WoP
    WoD[64:128, 256:512] = WoP
    shared = {"W12": W12, "B1": b1a, "WH2": Wh2, "BH": bha, "WOD": WoD}
    for m in per_core:
        m.update(shared)
    return ks, perms, per_core, bias_full


def kernel(**inputs):
    from concourse import bass_utils

    ks, perms, in_maps, bias_full = _host_prep(
        inputs["x"], inputs["pos"], inputs["edge_index"],
        inputs["W1"], inputs["b1"], inputs["Wh"], inputs["bh"],
        inputs["Wo"], inputs["bo"])

    if ks not in _PROGRAM_CACHE:
        _PROGRAM_CACHE[ks] = _build_program(ks)
    nc = _PROGRAM_CACHE[ks]

    res = bass_utils.run_bass_kernel_spmd(nc, in_maps,
                                          core_ids=list(range(N_CORES)))
    cores = []
    for c, r in enumerate(res.results):
        o = r["OUT"]                                   # [128, 49*16] slot-major
        o = o.reshape(128, B_PER_CORE, OUT_CH).transpose(1, 0, 2)
        core_out = np.empty((B_PER_CORE, 128, OUT_CH), dtype=np.float32)
        core_out[perms[c]] = o                         # un-permute slots
        cores.append(core_out.reshape(CORE_NODES, OUT_CH))
    out = np.concatenate(cores, axis=0)                # [50176, 16]
    out = out + bias_full
    return np.ascontiguousarray(
        out[:N_POINTS].reshape(1, N_POINTS, OUT_CH).astype(np.float32))


# revision 17
# speedup vs baseline: 2.8656x; 1.0428x over previous
"""Trainium2 Bass kernel for IntegralTransform GNN message passing.

Strategy (dst-sharded, 8 cores, V5):
  - Node space padded to 50176 = 8 * 49 * 128. Core c owns nodes
    [c*6272, (c+1)*6272) = 49 buckets of 128 nodes. Host bins edges by dst
    bucket (stable sort) and sorts each core's buckets by edge count
    (descending) into 49 SLOTS; slot b's chunk count k_b is the max across
    the 8 cores, so one SPMD program fits all cores with minimal padding
    (~814 chunks/core vs 882 fixed). Host gathers pos[src]|pos[dst] and
    x[src] per slot, precomputes the per-chunk one-hot scatter matrices
    (bf16), and adds the Wo-bias term (xs_agg @ bo) after the device
    returns the pure message aggregation.
  - Edge stream is laid out two-chunks-deep: even chunks live on SBUF
    partitions 0-63, odd chunks on 64-127. L1/L2 run as two concurrent
    PE sub-tiles (col/row groups), so gelu processes 128 partitions per
    op. L3 runs one matmul per chunk PAIR: the two-deep h2 column block
    [128,128] is exactly the stacked operand for a block-diagonal
    rhs [[Wo,0],[0,Wo]] [128,512], yielding hp for both chunks in one
    [128,512] PSUM bank (one LDWEIGHTS per two chunks). Odd slot sizes
    get a single-chunk tail matmul.
  - DVE multiplies each hp pair by xs (broadcast along o) in ONE op
    straight out of PSUM -> prod bf16 [128, 2, 16, 16]. Scatter is
    transposed: acc2T[n, (o,i)] += oh_c^T @ prod_c -- one N=256 matmul
    and one LDWEIGHTS (the one-hot) per chunk, a single per-slot
    accumulation group in one PSUM bank. A per-slot DVE tensor_reduce
    over i produces out[n, o] directly; one [128, 49*16] DMA returns it.
    No collectives; host un-permutes slots, concatenates, adds the bias.
"""

import numpy as np
import ml_dtypes

N_POINTS = 50000
N_PAD = 50176          # 8 * 49 * 128
N_CORES = 8
BUCKET = 128           # nodes per bucket
B_PER_CORE = 49
N_BUCKETS = N_PAD // BUCKET   # 392
CORE_NODES = B_PER_CORE * BUCKET  # 6272
IN_CH = 16
OUT_CH = 16
HID = 64
POS = 3

BF16 = ml_dtypes.bfloat16

_PROGRAM_CACHE = {}


def _build_program(ks):
    """Build + compile the per-core Bass program.

    ks = tuple of 49 per-slot chunk counts (shared across cores)."""
    import concourse.bacc as bacc
    import concourse.tile as tile
    import concourse.mybir as mybir

    f32 = mybir.dt.float32
    bf16 = mybir.dt.bfloat16

    nblks = [(k + 1) // 2 for k in ks]       # 128-col blocks in two-deep layout
    hoffs = np.concatenate([[0], np.cumsum([n * 128 for n in nblks])])
    coffs = np.concatenate([[0], np.cumsum(ks)])
    total_half = int(hoffs[-1])
    total_chunks = int(coffs[-1])
    Kmax = max(ks)
    Hmax = max(nblks) * 128

    nc = bacc.Bacc("TRN2", target_bir_lowering=False, debug=False)

    PT = nc.dram_tensor("PT", [12, total_half], bf16, kind="ExternalInput")
    XSB = nc.dram_tensor("XSB", [128, total_chunks * 16], bf16,
                         kind="ExternalInput")
    OH = nc.dram_tensor("OH", [128, total_chunks * 128], bf16,
                        kind="ExternalInput")
    W12 = nc.dram_tensor("W12", [128, HID], bf16, kind="ExternalInput")
    B1 = nc.dram_tensor("B1", [128, 1], f32, kind="ExternalInput")
    WH2 = nc.dram_tensor("WH2", [128, HID], bf16, kind="ExternalInput")
    BH = nc.dram_tensor("BH", [128, 1], f32, kind="ExternalInput")
    WOD = nc.dram_tensor("WOD", [128, 512], bf16, kind="ExternalInput")
    OUT = nc.dram_tensor("OUT", [128, B_PER_CORE * 16], f32,
                         kind="ExternalOutput")

    Gelu = mybir.ActivationFunctionType.Gelu
    MUL = mybir.AluOpType.mult
    ADD = mybir.AluOpType.add
    X = mybir.AxisListType.X

    with tile.TileContext(nc) as tc:
        with tc.tile_pool(name="const", bufs=1) as cp, \
             tc.tile_pool(name="io", bufs=2) as io, \
             tc.tile_pool(name="hh", bufs=2) as hh, \
             tc.tile_pool(name="wk", bufs=4) as wk, \
             tc.tile_pool(name="psMLP", bufs=2, space="PSUM") as psMLP, \
             tc.tile_pool(name="psH", bufs=4, space="PSUM") as psH, \
             tc.tile_pool(name="psAcc", bufs=2, space="PSUM") as psAcc:

            # --- constants ---
            w12_t = cp.tile([128, HID], bf16)
            nc.sync.dma_start(out=w12_t[:], in_=W12[:])
            b1_t = cp.tile([128, 1], f32)
            nc.sync.dma_start(out=b1_t[:], in_=B1[:])
            wh2_t = cp.tile([128, HID], bf16)
            nc.sync.dma_start(out=wh2_t[:], in_=WH2[:])
            bh_t = cp.tile([128, 1], f32)
            nc.sync.dma_start(out=bh_t[:], in_=BH[:])
            wod_t = cp.tile([128, 512], bf16)
            nc.sync.dma_start(out=wod_t[:], in_=WOD[:])

            fin_t = cp.tile([128, B_PER_CORE * 16], f32)

            for b in range(B_PER_CORE):
                k = ks[b]
                npairs, tail = k // 2, k % 2
                nblk = npairs + tail
                Hb = nblk * 128
                ho = int(hoffs[b])
                co = int(coffs[b])

                pt_t = io.tile([128, Hmax], bf16, tag="pt")
                nc.sync.dma_start(out=pt_t[0:6, 0:Hb],
                                  in_=PT[0:6, ho:ho + Hb])
                nc.sync.dma_start(out=pt_t[64:70, 0:Hb],
                                  in_=PT[6:12, ho:ho + Hb])
                xsb_t = io.tile([128, Kmax, 16], bf16, tag="xsb")
                nc.sync.dma_start(out=xsb_t[:, 0:k, :],
                                  in_=XSB[:, co * 16:(co + k) * 16])
                oh_t = io.tile([128, Kmax * 128], bf16, tag="oh")
                nc.sync.dma_start(out=oh_t[:, 0:k * 128],
                                  in_=OH[:, co * 128:(co + k) * 128])

                # ---- L1: two concurrent col-group tiles, gelu on 128p ----
                h1_t = hh.tile([128, Hmax], bf16, tag="h1")
                h2_t = hh.tile([128, Hmax], bf16, tag="h2")
                for s0 in range(0, Hb, 512):
                    w = min(512, Hb - s0)
                    p1 = psMLP.tile([128, 512], f32, tag="ps_mlp")
                    nc.tensor.matmul(p1[0:64, 0:w], lhsT=w12_t[0:6, :],
                                     rhs=pt_t[0:6, s0:s0 + w],
                                     start=True, stop=True)
                    nc.tensor.matmul(p1[64:128, 0:w], lhsT=w12_t[64:70, :],
                                     rhs=pt_t[64:70, s0:s0 + w],
                                     start=True, stop=True)
                    nc.scalar.activation(h1_t[:, s0:s0 + w], p1[:, 0:w], Gelu,
                                         bias=b1_t[:], scale=1.0)
                # ---- L2: two concurrent (row,col) diagonal tiles ----
                for s0 in range(0, Hb, 512):
                    w = min(512, Hb - s0)
                    p2 = psMLP.tile([128, 512], f32, tag="ps_mlp")
                    nc.tensor.matmul(p2[0:64, 0:w], lhsT=wh2_t[0:64, :],
                                     rhs=h1_t[0:64, s0:s0 + w],
                                     start=True, stop=True)
                    nc.tensor.matmul(p2[64:128, 0:w], lhsT=wh2_t[64:128, :],
                                     rhs=h1_t[64:128, s0:s0 + w],
                                     start=True, stop=True)
                    nc.scalar.activation(h2_t[:, s0:s0 + w], p2[:, 0:w], Gelu,
                                         bias=bh_t[:], scale=1.0)

                # ---- per-pair hp (block-diag) + mult + skewed scatter ----
                acc2 = psAcc.tile([128, 256], f32, tag="acc2")
                prev = None

                def emit_scatter(chunks, prod2, first, last):
                    pr = prod2[:].rearrange("p c o i -> p (c o i)")
                    for idx, c in enumerate(chunks):
                        nc.tensor.matmul(
                            acc2[:], lhsT=oh_t[:, c * 128:(c + 1) * 128],
                            rhs=pr[:, idx * 256:(idx + 1) * 256],
                            start=(first and idx == 0),
                            stop=(last and idx == len(chunks) - 1))

                for m in range(nblk):
                    is_tail = (tail == 1 and m == nblk - 1)
                    hpP = psH.tile([128, 512], f32, tag="hpP")
                    prod2 = wk.tile([128, 2, 16, 16], bf16, tag="prod")
                    if not is_tail:
                        nc.tensor.matmul(hpP[:],
                                         lhsT=h2_t[:, m * 128:(m + 1) * 128],
                                         rhs=wod_t[:], start=True, stop=True)
                        xs_b = xsb_t[:, 2 * m:2 * m + 2, :].unsqueeze(
                            2).to_broadcast([128, 2, 16, 16])
                        nc.vector.tensor_tensor(
                            out=prod2[:],
                            in0=hpP[:].rearrange("p (c o i) -> p c o i",
                                                 c=2, o=16, i=16),
                            in1=xs_b, op=MUL)
                        chunks = [2 * m, 2 * m + 1]
                    else:
                        nc.tensor.matmul(hpP[:, 0:256],
                                         lhsT=h2_t[0:64, m * 128:(m + 1) * 128],
                                         rhs=wod_t[0:64, 0:256],
                                         start=True, stop=True)
                        xs_b = xsb_t[:, k - 1:k, :].unsqueeze(
                            2).to_broadcast([128, 1, 16, 16])
                        nc.vector.tensor_tensor(
                            out=prod2[:, 0:1],
                            in0=hpP[:, 0:256].rearrange(
                                "p (c o i) -> p c o i", c=1, o=16, i=16),
                            in1=xs_b, op=MUL)
                        chunks = [k - 1]
                    if prev is not None:
                        emit_scatter(prev[0], prev[1], prev[2], False)
                    prev = (chunks, prod2, m == 0)
                emit_scatter(prev[0], prev[1], prev[2], True)

                # out[n, o] = sum_i acc2T[n, (o,i)]
                nc.vector.tensor_reduce(
                    out=fin_t[:, b * 16:(b + 1) * 16],
                    in_=acc2[:].rearrange("p (o i) -> p o i", o=16, i=16),
                    axis=X, op=ADD)

            nc.sync.dma_start(out=OUT[:], in_=fin_t[:])

    nc.compile()
    return nc


def _host_prep(x, pos, edge_index, W1, b1, Wh, bh, Wo, bo):
    """Bin edges by dst bucket, sort buckets into slots, gather, pad."""
    x_flat = np.ascontiguousarray(x.reshape(-1, IN_CH).astype(np.float32))
    pos = np.ascontiguousarray(pos.astype(np.float32))
    src = np.asarray(edge_index[0], dtype=np.int64)
    dst = np.asarray(edge_index[1], dtype=np.int64)
    E = src.shape[0]

    bucket = (dst >> 7).astype(np.int64)          # 0..391
    order = np.argsort(bucket, kind="stable")     # edge ids sorted by bucket
    cnt = np.bincount(bucket, minlength=N_BUCKETS)
    starts = np.zeros(N_BUCKETS, dtype=np.int64)
    starts[1:] = np.cumsum(cnt)[:-1]

    cnt_pc = cnt.reshape(N_CORES, B_PER_CORE)
    perms = np.argsort(-cnt_pc, axis=1, kind="stable")   # slot -> local bucket
    sorted_cnt = np.take_along_axis(cnt_pc, perms, axis=1)
    slot_max = sorted_cnt.max(axis=0)
    ks = tuple(int(v) for v in np.maximum(1, np.ceil(slot_max / 128))
               .astype(np.int64))

    nblks = [(k + 1) // 2 for k in ks]
    hoffs = np.concatenate([[0], np.cumsum([n * 128 for n in nblks])])
    coffs = np.concatenate([[0], np.cumsum(ks)])
    total_half = int(hoffs[-1])
    total_chunks = int(coffs[-1])

    # host-side bias term
    e_src_all = src[order]
    e_dst_all = dst[order]
    xs_agg = np.zeros((N_PAD, IN_CH), dtype=np.float32)
    np.add.at(xs_agg, e_dst_all, x_flat[e_src_all])
    bo16 = np.asarray(bo, dtype=np.float32).reshape(IN_CH, OUT_CH)
    bias_full = xs_agg @ bo16                      # [N_PAD, 16]

    per_core = []
    for c in range(N_CORES):
        PT2 = np.zeros((12, total_half), dtype=np.float32)
        XS2 = np.zeros((128, total_chunks, 16), dtype=np.float32)
        OH2 = np.zeros((128, total_chunks, 128), dtype=BF16)
        for b in range(B_PER_CORE):
            k = ks[b]
            nblk = nblks[b]
            ho = int(hoffs[b])
            g = c * B_PER_CORE + int(perms[c][b])  # global bucket id
            n = int(cnt_pc[c][perms[c][b]])
            if n == 0:
                continue
            eids = order[starts[g]:starts[g] + n]
            es, ed = src[eids], dst[eids]
            pe6 = np.concatenate([pos[es], pos[ed]], axis=1)   # [n, 6]
            xse = x_flat[es]                                   # [n, 16]
            dl = (ed - (g << 7)).astype(np.int64)
            ch = np.arange(n) // 128                           # chunk in slot
            rw = np.arange(n) % 128                            # row (edge lane)
            # PT two-deep: chunk 2m -> rows 0-5 block m; 2m+1 -> rows 6-11
            colh = (ch // 2) * 128 + rw
            hi = (ch % 2) * 6
            PT2[hi, ho + colh] = pe6[:, 0]
            PT2[hi + 1, ho + colh] = pe6[:, 1]
            PT2[hi + 2, ho + colh] = pe6[:, 2]
            PT2[hi + 3, ho + colh] = pe6[:, 3]
            PT2[hi + 4, ho + colh] = pe6[:, 4]
            PT2[hi + 5, ho + colh] = pe6[:, 5]
            XS2[rw, int(coffs[b]) + ch] = xse
            OH2[rw, int(coffs[b]) + ch, dl] = 1
        per_core.append({
            "PT": PT2.astype(BF16),
            "XSB": np.ascontiguousarray(XS2.reshape(128, total_chunks * 16)
                                        ).astype(BF16),
            "OH": np.ascontiguousarray(OH2.reshape(128, total_chunks * 128)),
        })

    # weights (shared across cores)
    W1a = np.asarray(W1, dtype=BF16)                                # [6, 64]
    W12 = np.zeros((128, HID), dtype=BF16)
    W12[0:6] = W1a
    W12[64:70] = W1a
    b1a = np.tile(np.asarray(b1, dtype=np.float32).reshape(HID, 1), (2, 1))
    Wha = np.asarray(Wh, dtype=BF16)                                # [64, 64]
    Wh2 = np.vstack([Wha, Wha])                                     # [128, 64]
    bha = np.tile(np.asarray(bh, dtype=np.float32).reshape(HID, 1), (2, 1))
    WoP = np.asarray(Wo, dtype=np.float32).reshape(HID, IN_CH, OUT_CH)
    WoP = np.ascontiguousarray(WoP.transpose(0, 2, 1)).reshape(HID, 256)
    WoP = WoP.astype(BF16)                                          # [64,(o,i)]
    WoD = np.zeros((128, 512), dtype=BF16)
    WoD[0:64, 0:256] = WoP
    WoD[64:128, 256:512] = WoP
    shared = {"W12": W12, "B1": b1a, "WH2": Wh2, "BH": bha, "WOD": WoD}
    for m in per_core:
        m.update(shared)
    return ks, perms, per_core, bias_full


def kernel(**inputs):
    from concourse import bass_utils

    ks, perms, in_maps, bias_full = _host_prep(
        inputs["x"], inputs["pos"], inputs["edge_index"],
        inputs["W1"], inputs["b1"], inputs["Wh"], inputs["bh"],
        inputs["Wo"], inputs["bo"])

    if ks not in _PROGRAM_CACHE:
        _PROGRAM_CACHE[ks] = _build_program(ks)
    nc = _PROGRAM_CACHE[ks]

    res = bass_utils.run_bass_kernel_spmd(nc, in_maps,
                                          core_ids=list(range(N_CORES)))
    cores = []
    for c, r in enumerate(res.results):
        o = r["OUT"]                                   # [128, 49*16] slot-major
        o = o.reshape(128, B_PER_CORE, OUT_CH).transpose(1, 0, 2)
        core_out = np.empty((B_PER_CORE, 128, OUT_CH), dtype=np.float32)
        core_out[perms[c]] = o                         # un-permute slots
        cores.append(core_out.reshape(CORE_NODES, OUT_CH))
    out = np.concatenate(cores, axis=0)                # [50176, 16]
    out = out + bias_full
    return np.ascontiguousarray(
        out[:N_POINTS].reshape(1, N_POINTS, OUT_CH).astype(np.float32))


# revision 18
# speedup vs baseline: 2.8906x; 1.0087x over previous
"""Trainium2 Bass kernel for IntegralTransform GNN message passing.

Strategy (dst-sharded, 8 cores, V5):
  - Node space padded to 50176 = 8 * 49 * 128. Core c owns nodes
    [c*6272, (c+1)*6272) = 49 buckets of 128 nodes. Host bins edges by dst
    bucket (stable sort) and sorts each core's buckets by edge count
    (descending) into 49 SLOTS; slot b's chunk count k_b is the max across
    the 8 cores, so one SPMD program fits all cores with minimal padding
    (~814 chunks/core vs 882 fixed). Host gathers pos[src]|pos[dst] and
    x[src] per slot, precomputes the per-chunk one-hot scatter matrices
    (bf16), and adds the Wo-bias term (xs_agg @ bo) after the device
    returns the pure message aggregation.
  - Edge stream is laid out two-chunks-deep: even chunks live on SBUF
    partitions 0-63, odd chunks on 64-127. L1/L2 run as two concurrent
    PE sub-tiles (col/row groups), so gelu processes 128 partitions per
    op. L3 runs one matmul per chunk PAIR: the two-deep h2 column block
    [128,128] is exactly the stacked operand for a block-diagonal
    rhs [[Wo,0],[0,Wo]] [128,512], yielding hp for both chunks in one
    [128,512] PSUM bank (one LDWEIGHTS per two chunks). Odd slot sizes
    get a single-chunk tail matmul.
  - DVE multiplies each hp pair by xs (broadcast along o) in ONE op
    straight out of PSUM -> prod bf16 [128, 2, 16, 16]. Scatter is
    transposed: acc2T[n, (o,i)] += oh_c^T @ prod_c -- one N=256 matmul
    and one LDWEIGHTS (the one-hot) per chunk, a single per-slot
    accumulation group in one PSUM bank. A per-slot DVE tensor_reduce
    over i produces out[n, o] directly; one [128, 49*16] DMA returns it.
    No collectives; host un-permutes slots, concatenates, adds the bias.
"""

import numpy as np
import ml_dtypes

N_POINTS = 50000
N_PAD = 50176          # 8 * 49 * 128
N_CORES = 8
BUCKET = 128           # nodes per bucket
B_PER_CORE = 49
N_BUCKETS = N_PAD // BUCKET   # 392
CORE_NODES = B_PER_CORE * BUCKET  # 6272
IN_CH = 16
OUT_CH = 16
HID = 64
POS = 3

BF16 = ml_dtypes.bfloat16

_PROGRAM_CACHE = {}


def _build_program(ks):
    """Build + compile the per-core Bass program.

    ks = tuple of 49 per-slot chunk counts (shared across cores)."""
    import concourse.bacc as bacc
    import concourse.tile as tile
    import concourse.mybir as mybir

    f32 = mybir.dt.float32
    bf16 = mybir.dt.bfloat16

    nblks = [(k + 1) // 2 for k in ks]       # 128-col blocks in two-deep layout
    hoffs = np.concatenate([[0], np.cumsum([n * 128 for n in nblks])])
    coffs = np.concatenate([[0], np.cumsum(ks)])
    total_half = int(hoffs[-1])
    total_chunks = int(coffs[-1])
    Kmax = max(ks)
    Hmax = max(nblks) * 128

    nc = bacc.Bacc("TRN2", target_bir_lowering=False, debug=False)

    PT = nc.dram_tensor("PT", [12, total_half], bf16, kind="ExternalInput")
    XSB = nc.dram_tensor("XSB", [128, total_chunks * 16], bf16,
                         kind="ExternalInput")
    OH = nc.dram_tensor("OH", [128, total_chunks * 128], bf16,
                        kind="ExternalInput")
    W12 = nc.dram_tensor("W12", [128, HID], bf16, kind="ExternalInput")
    B1 = nc.dram_tensor("B1", [128, 1], f32, kind="ExternalInput")
    WH2 = nc.dram_tensor("WH2", [128, HID], bf16, kind="ExternalInput")
    BH = nc.dram_tensor("BH", [128, 1], f32, kind="ExternalInput")
    WOD = nc.dram_tensor("WOD", [128, 512], bf16, kind="ExternalInput")
    OUT = nc.dram_tensor("OUT", [128, B_PER_CORE * 16], f32,
                         kind="ExternalOutput")

    Gelu = mybir.ActivationFunctionType.Gelu
    MUL = mybir.AluOpType.mult
    ADD = mybir.AluOpType.add
    X = mybir.AxisListType.X

    with tile.TileContext(nc) as tc:
        with tc.tile_pool(name="const", bufs=1) as cp, \
             tc.tile_pool(name="io", bufs=2) as io, \
             tc.tile_pool(name="hh", bufs=2) as hh, \
             tc.tile_pool(name="wk", bufs=4) as wk, \
             tc.tile_pool(name="psMLP", bufs=2, space="PSUM") as psMLP, \
             tc.tile_pool(name="psH", bufs=4, space="PSUM") as psH, \
             tc.tile_pool(name="psAcc", bufs=2, space="PSUM") as psAcc:

            # --- constants ---
            w12_t = cp.tile([128, HID], bf16)
            nc.sync.dma_start(out=w12_t[:], in_=W12[:])
            b1_t = cp.tile([128, 1], f32)
            nc.sync.dma_start(out=b1_t[:], in_=B1[:])
            wh2_t = cp.tile([128, HID], bf16)
            nc.sync.dma_start(out=wh2_t[:], in_=WH2[:])
            bh_t = cp.tile([128, 1], f32)
            nc.sync.dma_start(out=bh_t[:], in_=BH[:])
            wod_t = cp.tile([128, 512], bf16)
            nc.sync.dma_start(out=wod_t[:], in_=WOD[:])

            fin_t = cp.tile([128, B_PER_CORE * 16], f32)

            for b in range(B_PER_CORE):
                k = ks[b]
                npairs, tail = k // 2, k % 2
                nblk = npairs + tail
                Hb = nblk * 128
                ho = int(hoffs[b])
                co = int(coffs[b])

                pt_t = io.tile([128, Hmax], bf16, tag="pt")
                nc.sync.dma_start(out=pt_t[0:6, 0:Hb],
                                  in_=PT[0:6, ho:ho + Hb])
                nc.sync.dma_start(out=pt_t[64:70, 0:Hb],
                                  in_=PT[6:12, ho:ho + Hb])
                xsb_t = io.tile([128, Kmax, 16], bf16, tag="xsb")
                nc.sync.dma_start(out=xsb_t[:, 0:k, :],
                                  in_=XSB[:, co * 16:(co + k) * 16])
                oh_t = io.tile([128, Kmax * 128], bf16, tag="oh")
                nc.sync.dma_start(out=oh_t[:, 0:k * 128],
                                  in_=OH[:, co * 128:(co + k) * 128])

                # ---- L1: two concurrent col-group tiles, gelu on 128p ----
                h1_t = hh.tile([128, Hmax], bf16, tag="h1")
                h2_t = hh.tile([128, Hmax], bf16, tag="h2")
                for s0 in range(0, Hb, 512):
                    w = min(512, Hb - s0)
                    p1 = psMLP.tile([128, 512], f32, tag="ps_mlp")
                    nc.tensor.matmul(p1[0:64, 0:w], lhsT=w12_t[0:6, :],
                                     rhs=pt_t[0:6, s0:s0 + w],
                                     start=True, stop=True)
                    nc.tensor.matmul(p1[64:128, 0:w], lhsT=w12_t[64:70, :],
                                     rhs=pt_t[64:70, s0:s0 + w],
                                     start=True, stop=True)
                    nc.scalar.activation(h1_t[:, s0:s0 + w], p1[:, 0:w], Gelu,
                                         bias=b1_t[:], scale=1.0)
                # ---- L2: two concurrent (row,col) diagonal tiles ----
                for s0 in range(0, Hb, 512):
                    w = min(512, Hb - s0)
                    p2 = psMLP.tile([128, 512], f32, tag="ps_mlp")
                    nc.tensor.matmul(p2[0:64, 0:w], lhsT=wh2_t[0:64, :],
                                     rhs=h1_t[0:64, s0:s0 + w],
                                     start=True, stop=True)
                    nc.tensor.matmul(p2[64:128, 0:w], lhsT=wh2_t[64:128, :],
                                     rhs=h1_t[64:128, s0:s0 + w],
                                     start=True, stop=True)
                    nc.scalar.activation(h2_t[:, s0:s0 + w], p2[:, 0:w], Gelu,
                                         bias=bh_t[:], scale=1.0)

                # ---- per-pair hp (block-diag) + mult + skewed scatter ----
                acc2 = psAcc.tile([128, 256], f32, tag="acc2")
                pend = []

                def emit_scatter(chunks, prod2, first, last):
                    pr = prod2[:].rearrange("p c o i -> p (c o i)")
                    for idx, c in enumerate(chunks):
                        nc.tensor.matmul(
                            acc2[:], lhsT=oh_t[:, c * 128:(c + 1) * 128],
                            rhs=pr[:, idx * 256:(idx + 1) * 256],
                            start=(first and idx == 0),
                            stop=(last and idx == len(chunks) - 1))

                for m in range(nblk):
                    is_tail = (tail == 1 and m == nblk - 1)
                    hpP = psH.tile([128, 512], f32, tag="hpP")
                    prod2 = wk.tile([128, 2, 16, 16], bf16, tag="prod")
                    if not is_tail:
                        nc.tensor.matmul(hpP[:],
                                         lhsT=h2_t[:, m * 128:(m + 1) * 128],
                                         rhs=wod_t[:], start=True, stop=True)
                        xs_b = xsb_t[:, 2 * m:2 * m + 2, :].unsqueeze(
                            2).to_broadcast([128, 2, 16, 16])
                        nc.vector.tensor_tensor(
                            out=prod2[:],
                            in0=hpP[:].rearrange("p (c o i) -> p c o i",
                                                 c=2, o=16, i=16),
                            in1=xs_b, op=MUL)
                        chunks = [2 * m, 2 * m + 1]
                    else:
                        nc.tensor.matmul(hpP[:, 0:256],
                                         lhsT=h2_t[0:64, m * 128:(m + 1) * 128],
                                         rhs=wod_t[0:64, 0:256],
                                         start=True, stop=True)
                        xs_b = xsb_t[:, k - 1:k, :].unsqueeze(
                            2).to_broadcast([128, 1, 16, 16])
                        nc.vector.tensor_tensor(
                            out=prod2[:, 0:1],
                            in0=hpP[:, 0:256].rearrange(
                                "p (c o i) -> p c o i", c=1, o=16, i=16),
                            in1=xs_b, op=MUL)
                        chunks = [k - 1]
                    if len(pend) == 2:
                        e = pend.pop(0)
                        emit_scatter(e[0], e[1], e[2], False)
                    pend.append((chunks, prod2, m == 0))
                for j, e in enumerate(pend):
                    emit_scatter(e[0], e[1], e[2], j == len(pend) - 1)

                # out[n, o] = sum_i acc2T[n, (o,i)]
                nc.vector.tensor_reduce(
                    out=fin_t[:, b * 16:(b + 1) * 16],
                    in_=acc2[:].rearrange("p (o i) -> p o i", o=16, i=16),
                    axis=X, op=ADD)

            nc.sync.dma_start(out=OUT[:], in_=fin_t[:])

    nc.compile()
    return nc


def _host_prep(x, pos, edge_index, W1, b1, Wh, bh, Wo, bo):
    """Bin edges by dst bucket, sort buckets into slots, gather, pad."""
    x_flat = np.ascontiguousarray(x.reshape(-1, IN_CH).astype(np.float32))
    pos = np.ascontiguousarray(pos.astype(np.float32))
    src = np.asarray(edge_index[0], dtype=np.int64)
    dst = np.asarray(edge_index[1], dtype=np.int64)
    E = src.shape[0]

    bucket = (dst >> 7).astype(np.int64)          # 0..391
    order = np.argsort(bucket, kind="stable")     # edge ids sorted by bucket
    cnt = np.bincount(bucket, minlength=N_BUCKETS)
    starts = np.zeros(N_BUCKETS, dtype=np.int64)
    starts[1:] = np.cumsum(cnt)[:-1]

    cnt_pc = cnt.reshape(N_CORES, B_PER_CORE)
    perms = np.argsort(-cnt_pc, axis=1, kind="stable")   # slot -> local bucket
    sorted_cnt = np.take_along_axis(cnt_pc, perms, axis=1)
    slot_max = sorted_cnt.max(axis=0)
    ks = tuple(int(v) for v in np.maximum(1, np.ceil(slot_max / 128))
               .astype(np.int64))

    nblks = [(k + 1) // 2 for k in ks]
    hoffs = np.concatenate([[0], np.cumsum([n * 128 for n in nblks])])
    coffs = np.concatenate([[0], np.cumsum(ks)])
    total_half = int(hoffs[-1])
    total_chunks = int(coffs[-1])

    # host-side bias term
    e_src_all = src[order]
    e_dst_all = dst[order]
    xs_agg = np.zeros((N_PAD, IN_CH), dtype=np.float32)
    np.add.at(xs_agg, e_dst_all, x_flat[e_src_all])
    bo16 = np.asarray(bo, dtype=np.float32).reshape(IN_CH, OUT_CH)
    bias_full = xs_agg @ bo16                      # [N_PAD, 16]

    per_core = []
    for c in range(N_CORES):
        PT2 = np.zeros((12, total_half), dtype=np.float32)
        XS2 = np.zeros((128, total_chunks, 16), dtype=np.float32)
        OH2 = np.zeros((128, total_chunks, 128), dtype=BF16)
        for b in range(B_PER_CORE):
            k = ks[b]
            nblk = nblks[b]
            ho = int(hoffs[b])
            g = c * B_PER_CORE + int(perms[c][b])  # global bucket id
            n = int(cnt_pc[c][perms[c][b]])
            if n == 0:
                continue
            eids = order[starts[g]:starts[g] + n]
            es, ed = src[eids], dst[eids]
            pe6 = np.concatenate([pos[es], pos[ed]], axis=1)   # [n, 6]
            xse = x_flat[es]                                   # [n, 16]
            dl = (ed - (g << 7)).astype(np.int64)
            ch = np.arange(n) // 128                           # chunk in slot
            rw = np.arange(n) % 128                            # row (edge lane)
            # PT two-deep: chunk 2m -> rows 0-5 block m; 2m+1 -> rows 6-11
            colh = (ch // 2) * 128 + rw
            hi = (ch % 2) * 6
            PT2[hi, ho + colh] = pe6[:, 0]
            PT2[hi + 1, ho + colh] = pe6[:, 1]
            PT2[hi + 2, ho + colh] = pe6[:, 2]
            PT2[hi + 3, ho + colh] = pe6[:, 3]
            PT2[hi + 4, ho + colh] = pe6[:, 4]
            PT2[hi + 5, ho + colh] = pe6[:, 5]
            XS2[rw, int(coffs[b]) + ch] = xse
            OH2[rw, int(coffs[b]) + ch, dl] = 1
        per_core.append({
            "PT": PT2.astype(BF16),
            "XSB": np.ascontiguousarray(XS2.reshape(128, total_chunks * 16)
                                        ).astype(BF16),
            "OH": np.ascontiguousarray(OH2.reshape(128, total_chunks * 128)),
        })

    # weights (shared across cores)
    W1a = np.asarray(W1, dtype=BF16)                                # [6, 64]
    W12 = np.zeros((128, HID), dtype=BF16)
    W12[0:6] = W1a
    W12[64:70] = W1a
    b1a = np.tile(np.asarray(b1, dtype=np.float32).reshape(HID, 1), (2, 1))
    Wha = np.asarray(Wh, dtype=BF16)                                # [64, 64]
    Wh2 = np.vstack([Wha, Wha])                                     # [128, 64]
    bha = np.tile(np.asarray(bh, dtype=np.float32).reshape(HID, 1), (2, 1))
    WoP = np.asarray(Wo, dtype=np.float32).reshape(HID, IN_CH, OUT_CH)
    WoP = np.ascontiguousarray(WoP.transpose(0, 2, 1)).reshape(HID, 256)
    WoP = WoP.astype(BF16)                                          # [64,(o,i)]
    WoD = np.zeros((128, 512), dtype=BF16)
    WoD[0:64, 0:256] = WoP
    WoD[64:128, 256:512] = WoP
    shared = {"W12": W12, "B1": b1a, "WH2": Wh2, "BH": bha, "WOD": WoD}
    for m in per_core:
        m.update(shared)
    return ks, perms, per_core, bias_full


def kernel(**inputs):
    from concourse import bass_utils

    ks, perms, in_maps, bias_full = _host_prep(
        inputs["x"], inputs["pos"], inputs["edge_index"],
        inputs["W1"], inputs["b1"], inputs["Wh"], inputs["bh"],
        inputs["Wo"], inputs["bo"])

    if ks not in _PROGRAM_CACHE:
        _PROGRAM_CACHE[ks] = _build_program(ks)
    nc = _PROGRAM_CACHE[ks]

    res = bass_utils.run_bass_kernel_spmd(nc, in_maps,
                                          core_ids=list(range(N_CORES)))
    cores = []
    for c, r in enumerate(res.results):
        o = r["OUT"]                                   # [128, 49*16] slot-major
        o = o.reshape(128, B_PER_CORE, OUT_CH).transpose(1, 0, 2)
        core_out = np.empty((B_PER_CORE, 128, OUT_CH), dtype=np.float32)
        core_out[perms[c]] = o                         # un-permute slots
        cores.append(core_out.reshape(CORE_NODES, OUT_CH))
    out = np.concatenate(cores, axis=0)                # [50176, 16]
    out = out + bias_full
    return np.ascontiguousarray(
        out[:N_POINTS].reshape(1, N_POINTS, OUT_CH).astype(np.float32))
